# revision 23
# baseline (speedup 1.0000x reference)
# Trainium2 Bass kernel for nn_Attention_35433480192757
#
# reference computation (b=4, c=128, h=w=64, n=h*w=4096):
#   GroupNorm(8, c) -> 1x1 conv qkv -> full [n, n] attention per batch
#   -> 1x1 conv proj -> residual add
#
# Sharding: 8 cores = 4 batches x 2 query-row halves. Each core computes the
# full k/v for its batch (cheap: the qkv matmuls are tiny) and attention for
# its 2048 query rows. Host-side, each core's x is column-PERMUTED so that
# its own query half occupies columns 0:2048 -- attention is invariant to
# the j-enumeration order, and this keeps the SPMD program identical across
# cores with no separate xq input.
#
# Per-core strategy (fp8 + DoubleRow PV + two-engine softmax exp):
#   - x kept as [c=128 partitions, n] fp32; GroupNorm folded into the qkv
#     weights (xn = x*s_c + t_c per channel, computed on device; rsqrt via
#     a vector-engine bit-trick seed + Newton).
#   - q,k,v are produced as fp8e4. QK^T runs as plain fp8 contraction-128
#     matmuls. PV and the softmax-denominator (ones) matmuls consume P as
#     fp8 DoubleRow pairs.
#   - The k bias cancels in softmax; the v bias is folded into the proj
#     bias (pb' = pb + Wp @ bv).
#   - exp(scores) is written to fp8e5 by BOTH the scalar engine (true exp)
#     and the vector engine (Schraudolph bit-trick), split by a static
#     per-pair schedule.
#   - The whole per-rep prologue (GroupNorm stats, weight folds, k/q/v
#     production) is SOFTWARE-PIPELINED one rep ahead: its instructions are
#     emitted interleaved into the previous rep's attention pair loops, so
#     its PSUM tiles slot into the "sc"/"vv" rotations mid-stream and the
#     scalar engine never drains at rep boundaries.

import numpy as np
from contextlib import ExitStack

import concourse.bass as bass
from concourse import bacc
import concourse.tile as tile
import concourse.mybir as mybir
from concourse.bass import ts
from concourse.bass_utils import run_bass_kernel_spmd

P = 128          # partitions == channels
C = 128
N = 4096         # sequence length (h*w) per batch
NH = 2048        # query rows per core
CH = 512         # free-dim chunk (one PSUM bank of fp32)
NCH = N // CH    # 8 column chunks of x
NQCH = NH // CH  # 4 column chunks of q
NJC = N // P     # 32 key chunks (contraction over j)
NG = NJC // 2    # 16 j-chunk pairs per i-block
NIB = NH // CH   # 4 i-blocks per core
NUM_GROUPS = 8
GSIZE = C // NUM_GROUPS
EPS = 1e-5
SCALE = float(C) ** -0.5

F32 = mybir.dt.float32
F32R = mybir.dt.float32r
FP8 = mybir.dt.float8e4
FP8W = mybir.dt.float8e5   # P matrix: wide-range fp8 (e5m2)
I8 = mybir.dt.int8
AOP = mybir.AluOpType
AFT = mybir.ActivationFunctionType
DR = mybir.MatmulPerfMode.DoubleRow

# Schraudolph exp for fp8e5 (bias 15, 2 mantissa bits):
#   fp8e5_bits(exp(x)) ~= trunc(x * 4*log2e + 60 + c). e5m2's range covers
#   exp of +-10 sigma scores, so no clamping or shifting is needed; c=0.494
#   zeroes the mean multiplicative bias of the truncation.
SCHRAU_A = 4 * 1.4426950408889634
SCHRAU_B = 60.0 + 0.494

# Per-i-block sets of j-chunk pairs whose exp runs on the vector engine
# (Schraudolph); the rest use the scalar engine's exp.
DVE_EXP = {
    0: (1, 3, 5, 7, 9),
    1: (0, 2, 4, 6, 9, 11, 13),
    2: (1, 3, 5, 7, 9, 11, 13, 15),
    3: (2, 5, 8, 11, 14),
}

QK_PM = None
import os as _os
if _os.environ.get("QK_DP"):
    QK_PM = mybir.MatmulPerfMode.DoublePixel
if _os.environ.get("DVE_EXP_CFG"):
    # e.g. "1,3,5,7,9,11,13|2,4,6,8,11,14|1,4,6,9,11,13|2,5,8,11,14"
    _parts = _os.environ["DVE_EXP_CFG"].split("|")
    DVE_EXP = {i: tuple(int(v) for v in p.split(",") if v != "")
               for i, p in enumerate(_parts)}


def _build_program(reps=1):
    nc = bacc.Bacc(trn_type="TRN2", num_devices=8)

    x_d = nc.dram_tensor("x", [P, N], F32R, kind="ExternalInput")
    wqT_d = nc.dram_tensor("wqT", [P, P], F32, kind="ExternalInput")
    wkT_d = nc.dram_tensor("wkT", [P, P], F32, kind="ExternalInput")
    wvT_d = nc.dram_tensor("wvT", [P, P], F32, kind="ExternalInput")
    wpT_d = nc.dram_tensor("wpT", [P, P], F32R, kind="ExternalInput")
    qkvb_d = nc.dram_tensor("qkvb", [P, 3], F32, kind="ExternalInput")
    pb_d = nc.dram_tensor("pb", [P, 1], F32, kind="ExternalInput")
    gnw_d = nc.dram_tensor("gnw", [P, 1], F32, kind="ExternalInput")
    gnb_d = nc.dram_tensor("gnb", [P, 1], F32, kind="ExternalInput")
    out_d = nc.dram_tensor("out", [P, NH], F32, kind="ExternalOutput")

    gmat_np = np.zeros((P, P), dtype=np.float32)
    for g in range(NUM_GROUPS):
        gmat_np[g * GSIZE:(g + 1) * GSIZE, g * GSIZE:(g + 1) * GSIZE] = 1.0 / GSIZE
    gmat_d = nc.inline_tensor(gmat_np, "gmat")

    with ExitStack() as ctx:
        tc = ctx.enter_context(tile.TileContext(nc))

        consts = ctx.enter_context(tc.tile_pool(name="consts", bufs=1))
        wfold = ctx.enter_context(tc.tile_pool(name="wfold", bufs=2))
        xpool = ctx.enter_context(tc.tile_pool(name="xpool", bufs=2))
        kqv = ctx.enter_context(tc.tile_pool(name="kqv", bufs=2))
        ptp = ctx.enter_context(tc.tile_pool(name="ptp", bufs=3))
        work = ctx.enter_context(tc.tile_pool(name="work", bufs=2))
        small = ctx.enter_context(tc.tile_pool(name="small", bufs=2))
        outp = ctx.enter_context(tc.tile_pool(name="outp", bufs=2))
        psb = ctx.enter_context(tc.tile_pool(name="psb", bufs=3, space="PSUM"))
        psv = psb
        psacc = ctx.enter_context(tc.tile_pool(name="psacc", bufs=1, space="PSUM"))
        pssum = ctx.enter_context(tc.tile_pool(name="pssum", bufs=1, space="PSUM"))

        env = dict(
            nc=nc, consts=consts, wfold=wfold, xpool=xpool, kqv=kqv, ptp=ptp,
            work=work, small=small, outp=outp, psb=psb, psv=psv, psacc=psacc,
            pssum=pssum, x_d=x_d, wqT_d=wqT_d, wkT_d=wkT_d, wvT_d=wvT_d,
            wpT_d=wpT_d, qkvb_d=qkvb_d, pb_d=pb_d, gnw_d=gnw_d, gnb_d=gnb_d,
            gmat_d=gmat_d, out_d=out_d,
        )

        _emit_consts(env)

        # rep 0 prologue emitted serially upfront (cold start).
        st0 = _prologue_steps(env, 0)
        for _, fn in st0["steps"]:
            fn()
        states = [st0]

        for r in range(reps):
            nxt = _prologue_steps(env, r + 1) if r + 1 < reps else None
            _emit_attention(env, states[r], nxt)
            if nxt is not None:
                states.append(nxt)

    nc.compile()
    return nc


def _emit_consts(env):
    """One-time loads: weights, biases, gmat, ones; PE warmup."""
    nc = env["nc"]
    consts = env["consts"]

    wq = consts.tile([P, P], F32, tag="wq", name="wq")
    nc.sync.dma_start(wq[:], env["wqT_d"].ap())
    wk = consts.tile([P, P], F32, tag="wk", name="wk")
    nc.sync.dma_start(wk[:], env["wkT_d"].ap())
    wv = consts.tile([P, P], F32, tag="wv", name="wv")
    nc.sync.dma_start(wv[:], env["wvT_d"].ap())
    wp = consts.tile([P, P], F32R, tag="wp", name="wp")
    nc.sync.dma_start(wp[:], env["wpT_d"].ap())
    qkvb = consts.tile([P, 3], F32, tag="qkvb", name="qkvb")
    nc.sync.dma_start(qkvb[:], env["qkvb_d"].ap())
    pb = consts.tile([P, 1], F32, tag="pb", name="pb")
    nc.sync.dma_start(pb[:], env["pb_d"].ap())
    gnw = consts.tile([P, 1], F32, tag="gnw", name="gnw")
    nc.sync.dma_start(gnw[:], env["gnw_d"].ap())
    gnb = consts.tile([P, 1], F32, tag="gnb", name="gnb")
    nc.sync.dma_start(gnb[:], env["gnb_d"].ap())
    ones8 = consts.tile([P, 2, P], FP8, tag="ones8", name="ones8")
    nc.gpsimd.memset(ones8[:], 1.0)
    # gmat last on the queue: the warmup matmul below absorbs the DMA-queue
    # semaphore wait once (walrus codegen allows only one sync-wait on a
    # self-loading fp32 matmul).
    gmat = consts.tile([P, P], F32, tag="gmat", name="gmat")
    nc.sync.dma_start(gmat[:], env["gmat_d"].ap())

    ps_w = env["psb"].tile([P, 8], F32, tag="sc", name="ps_warm")
    nc.tensor.matmul(ps_w[:, 0:2], lhsT=gmat[:], rhs=gmat[:, 0:2])

    env["wq"], env["wk"], env["wv"], env["wp"] = wq, wk, wv, wp
    env["qkvb"], env["pb"], env["gnw"], env["gnb"] = qkvb, pb, gnw, gnb
    env["gmat"], env["ones8"] = gmat, ones8


def _prologue_steps(env, rep):
    """Build the prologue for rep `rep` as a state dict + ordered list of
    emission callbacks (to run serially for rep 0, or interleaved into the
    previous rep's attention loops)."""
    nc = env["nc"]
    small, wfold, xpool, kqv = (env["small"], env["wfold"], env["xpool"],
                                env["kqv"])
    psb, psv = env["psb"], env["psv"]

    st = {}
    steps = []

    def add(name):
        def deco(fn):
            steps.append((name, fn))
            return fn
        return deco

    @add("xdma")
    def _():
        x_sb = xpool.tile([P, N], F32R, tag="x", name=f"x_sb{rep}")
        st["x"] = x_sb
        for s in range(NCH // 2):
            eng = nc.sync if s % 2 == 0 else nc.gpsimd
            eng.dma_start(x_sb[:, ts(s, 2 * CH)], env["x_d"].ap()[:, ts(s, 2 * CH)])
        st["stats"] = small.tile([P, NCH, 6], F32, tag="stats",
                                 name=f"stats{rep}")

    for _s in range(NCH):
        def _stats(s=_s):
            nc.vector.bn_stats(st["stats"][:, s, :], st["x"][:, ts(s, CH)])
        steps.append((f"stats{_s}", _stats))

    @add("aggr")
    def _():
        mv = small.tile([P, 2], F32, tag="mv", name=f"mv{rep}")
        nc.vector.bn_aggr(mv[:], st["stats"][:])
        # t2 = [mean_c, E[x^2]_c]; prep on the (idle) gpsimd engine so the
        # chain does not queue behind DVE exps.
        t2 = small.tile([P, 2], F32, tag="t2", name=f"t2{rep}")
        nc.vector.tensor_copy(t2[:, 0:1], mv[:, 0:1])
        nc.vector.scalar_tensor_tensor(t2[:, 1:2], mv[:, 0:1], mv[:, 0:1],
                                       mv[:, 1:2], AOP.mult, AOP.add)
        st["t2"] = t2

    @add("gnmm")
    def _():
        # group stats via block-diagonal averaging matrix; the tiny PSUM
        # tile borrows a slot of the "sc" pair rotation.
        ps_t = psb.tile([P, 8], F32, tag="sc", name=f"ps_gn{rep}")
        nc.tensor.matmul(ps_t[:, 0:2], lhsT=env["gmat"][:], rhs=st["t2"][:])
        gstat = small.tile([P, 2], F32, tag="gstat", name=f"gstat{rep}")
        nc.scalar.copy(gstat[:], ps_t[:, 0:2])
        st["gstat"] = gstat

    @add("rstd")
    def _():
        # whole chain on DVE in one batch (walrus allows no ALU ops on Pool)
        gstat = st["gstat"]
        varn = small.tile([P, 1], F32, tag="varn", name=f"varn{rep}")
        nc.vector.scalar_tensor_tensor(varn[:], gstat[:, 0:1], gstat[:, 0:1],
                                       gstat[:, 1:2], AOP.mult, AOP.subtract)
        vpos = small.tile([P, 1], F32, tag="vpos", name=f"vpos{rep}")
        nc.vector.tensor_scalar(vpos[:], varn[:], -1.0, EPS, AOP.mult, AOP.add)
        tsh = small.tile([P, 1], mybir.dt.int32, tag="tsh", name=f"tsh{rep}")
        nc.vector.tensor_scalar(tsh[:], vpos[:].bitcast(mybir.dt.int32), 1,
                                None, AOP.arith_shift_right)
        rstd = small.tile([P, 1], F32, tag="rstd", name=f"rstd{rep}")
        nc.vector.tensor_scalar(rstd[:].bitcast(mybir.dt.int32), tsh[:], -1,
                                0x5f3759df, AOP.mult, AOP.add)
        for it in range(2):
            nt = small.tile([P, 1], F32, tag="nt", name=f"nt{rep}_{it}")
            nc.vector.tensor_mul(nt[:], rstd[:], rstd[:])
            nc.vector.tensor_mul(nt[:], nt[:], vpos[:])
            nc.vector.tensor_scalar(nt[:], nt[:], -0.5, 1.5, AOP.mult, AOP.add)
            nc.vector.tensor_mul(rstd[:], rstd[:], nt[:])
        s_c = small.tile([P, 1], F32, tag="s_c", name=f"s_c{rep}")
        nc.vector.tensor_mul(s_c[:], rstd[:], env["gnw"][:])
        # t_n = mean_g*s_c - gn_bias = -t_c
        t_n = small.tile([P, 1], F32, tag="t_n", name=f"t_n{rep}")
        nc.vector.scalar_tensor_tensor(t_n[:], gstat[:, 0:1], s_c[:],
                                       env["gnb"][:], AOP.mult, AOP.subtract)
        st["s_c"], st["t_n"] = s_c, t_n

    @add("foldq")
    def _():
        wq_s = wfold.tile([P, P], F32R, tag="wq_s", name=f"wq_s{rep}")
        nc.vector.tensor_scalar_mul(wq_s[:], env["wq"][:], st["s_c"][:])
        st["wq_s"] = wq_s

    @add("foldk")
    def _():
        wk_s = wfold.tile([P, P], F32R, tag="wk_s", name=f"wk_s{rep}")
        nc.vector.tensor_scalar_mul(wk_s[:], env["wk"][:], st["s_c"][:])
        st["wk_s"] = wk_s

    @add("foldv")
    def _():
        wv_s = wfold.tile([P, 2, P], F32R, tag="wv_s", name=f"wv_s{rep}")
        nc.vector.tensor_scalar_mul(wv_s[:, 0, :], env["wv"][:], st["s_c"][:])
        nc.vector.tensor_scalar_mul(wv_s[:, 1, :], env["wv"][:], st["s_c"][:])
        st["wv_s"] = wv_s

    @add("bias")
    def _():
        # q bias (k bias cancels; v bias folds into the proj bias below)
        ps_b = psb.tile([P, 8], F32, tag="sc", name=f"ps_b{rep}")
        nc.tensor.matmul(ps_b[:, 0:1], lhsT=env["wq"][:], rhs=st["t_n"][:])
        nc.tensor.matmul(ps_b[:, 1:2], lhsT=env["wv"][:], rhs=st["t_n"][:])
        bq = small.tile([P, 1], F32, tag="bq", name=f"bq{rep}")
        nc.scalar.activation(bq[:], ps_b[:, 0:1], AFT.Identity,
                             bias=env["qkvb"][:, 0:1], scale=-1.0)
        bv = small.tile([P, 1], F32, tag="bv", name=f"bv{rep}")
        nc.scalar.activation(bv[:], ps_b[:, 1:2], AFT.Identity,
                             bias=env["qkvb"][:, 2:3], scale=-1.0)
        st["bq"], st["bv"] = bq, bv

    @add("pbf")
    def _():
        ps_p = psb.tile([P, 8], F32, tag="sc", name=f"ps_p{rep}")
        nc.tensor.matmul(ps_p[:, 0:1], lhsT=env["wp"][:].bitcast(F32),
                         rhs=st["bv"][:])
        pbf = small.tile([P, 1], F32, tag="pbf", name=f"pbf{rep}")
        nc.scalar.activation(pbf[:], ps_p[:, 0:1], AFT.Identity,
                             bias=env["pb"][:], scale=1.0)
        st["pbf"] = pbf
        st["kT8"] = kqv.tile([P, N], FP8, tag="kdr", name=f"kT8{rep}")
        st["qT8"] = kqv.tile([P, NH], FP8, tag="qdr", name=f"qT8{rep}")
        st["vnat"] = kqv.tile([P, NJC, P], FP8, tag="vnat", name=f"vnat{rep}")

    for _s in range(NCH):
        def _k(s=_s):
            pk = psb.tile([P, CH], F32, tag="sc", name=f"psk{rep}_{s}")
            nc.tensor.matmul(pk[:], lhsT=st["wk_s"][:], rhs=st["x"][:, ts(s, CH)])
            nc.scalar.copy(st["kT8"][:, ts(s, CH)], pk[:])
        steps.append((f"k{_s}", _k))

    for _s in range(NQCH):
        def _q(s=_s):
            # q carries the bias: scalar-engine activation applies it
            pq = psb.tile([P, CH], F32, tag="sc", name=f"psq{rep}_{s}")
            nc.tensor.matmul(pq[:], lhsT=st["wq_s"][:], rhs=st["x"][:, ts(s, CH)])
            nc.scalar.activation(st["qT8"][:, ts(s, CH)], pq[:], AFT.Identity,
                                 bias=st["bq"][:], scale=1.0)
        steps.append((f"q{_s}", _q))

    for _g in range(NJC // 4):
        def _v(q8=_g):
            # four chunks per PSUM tile, each duplicated twice (free=256
            # keeps fp32r at full rate); one copy reads the first replicas.
            pv = psv.tile([P, 4, 2, P], F32, tag="sc", name=f"psv{rep}_{q8}")
            for h in range(4):
                jc = 4 * q8 + h
                nc.tensor.matmul(pv[:, h, :, :],
                                 lhsT=st["x"][:, jc * P:(jc + 1) * P],
                                 rhs=st["wv_s"][:])
            nc.vector.tensor_copy(st["vnat"][:, 4 * q8:4 * q8 + 4, :],
                                  pv[:, :, 0, :])
        steps.append((f"v{_g}", _v))

    st["steps"] = steps
    return st


# Placement of next-rep prologue steps inside the current rep's attention:
# {(ib, pair_g): [step names]}. Steps not listed here run at their default
# position (appended after the pair loop of the listed block).
def _placement():
    pl = {}
    pl[(0, None)] = ["xdma"]                     # x DMA at ib0 entry
    # stats: all 8 chunks over ib0's back half (no DVE exps there)
    for i in range(NCH):
        pl[(0, 8 + i)] = [f"stats{i}"]
    pl[(1, 1)] = ["aggr"]
    pl[(1, 3)] = ["gnmm"]
    pl[(1, 5)] = ["rstd"]
    pl[(1, 8)] = ["foldq", "foldk"]
    pl[(1, 10)] = ["foldv"]
    pl[(1, 12)] = ["bias"]
    pl[(1, 14)] = ["pbf"]
    for i in range(NCH):                          # k production over ib2
        pl[(2, 4 + i)] = [f"k{i}"]
    for i in range(NQCH):                         # q production at ib2 end
        pl[(2, 12 + i)] = [f"q{i}"]
    # v production: 8 calls over ib3 pairs 0..14 (one every other pair)
    for i in range(NJC // 4):
        pl.setdefault((3, 2 * i), []).append(f"v{i}")
    return pl


PLACEMENT = _placement()


def _emit_attention(env, st, nxt):
    nc = env["nc"]
    ptp, work, outp = env["ptp"], env["work"], env["outp"]
    psb, psv, psacc, pssum = env["psb"], env["psv"], env["psacc"], env["pssum"]

    nxt_steps = dict(nxt["steps"]) if nxt is not None else {}
    emitted = set()

    def run_extra(ib, g):
        for name in PLACEMENT.get((ib, g), []) or []:
            fn = nxt_steps.get(name)
            if fn is not None and name not in emitted:
                emitted.add(name)
                fn()

    x_sb, kT8, qT8, vnat = st["x"], st["kT8"], st["qT8"], st["vnat"]
    wp, pbf = env["wp"], st["pbf"]

    for ib in range(NIB):
        if nxt is not None:
            run_extra(ib, None)
        PT = ptp.tile([P, NJC, CH], FP8W, tag="PT", name=f"PT{ib}")
        acc = psacc.tile([P, CH], F32, tag="acc", name=f"acc{ib}")
        sm = pssum.tile([P, CH], F32, tag="sp", name=f"sm{ib}")
        qblk = qT8[:, ts(ib, CH)]
        dve_pairs = DVE_EXP[ib]

        def emit_pv(g):
            pair = PT[:, 2 * g:2 * g + 2, :]
            nc.tensor.matmul(acc[:], lhsT=vnat[:, 2 * g:2 * g + 2, :],
                             rhs=pair, perf_mode=DR,
                             start=(g == 0), stop=(g == NG - 1),
                             skip_group_check=True)
            nc.tensor.matmul(sm[:], lhsT=env["ones8"][:], rhs=pair,
                             perf_mode=DR, start=(g == 0), stop=(g == NG - 1),
                             skip_group_check=True)

        for g in range(NG):
            dve_own = g in dve_pairs
            ps = psb.tile([P, 2, CH], F32, tag="sc", name=f"ps{ib}_{g}")
            for h in range(2):
                jc = 2 * g + h
                kslice = kT8[:, jc * P:(jc + 1) * P]
                nc.tensor.matmul(ps[:, h, :], lhsT=kslice, rhs=qblk,
                                 perf_mode=QK_PM, skip_group_check=True)
            if dve_own:
                nc.vector.tensor_scalar(PT[:, 2 * g:2 * g + 2, :].bitcast(I8),
                                        ps[:], SCHRAU_A * SCALE, SCHRAU_B,
                                        AOP.mult, AOP.add)
            run_extra(ib, g)
            if g > 3:
                emit_pv(g - 4)
            if not dve_own:
                nc.scalar.activation(PT[:, 2 * g:2 * g + 2, :], ps[:],
                                     AFT.Exp, scale=SCALE)
        emit_pv(NG - 4)
        emit_pv(NG - 3)
        emit_pv(NG - 2)
        emit_pv(NG - 1)

        # normalize and project
        recip = work.tile([P, CH], F32, tag="recip", name=f"recip{ib}")
        nc.vector.reciprocal_approx_fast(recip[:], sm[:])
        outn = work.tile([P, CH], F32R, tag="outn", name=f"outn{ib}")
        nc.vector.tensor_mul(outn[:], acc[:], recip[:])

        psp = pssum.tile([P, CH], F32, tag="sp", name=f"psp{ib}")
        nc.tensor.matmul(psp[:], lhsT=wp[:], rhs=outn[:])
        stage = outp.tile([P, CH], F32, tag="stage", name=f"stage{ib}")
        nc.vector.scalar_tensor_tensor(stage[:], psp[:], pbf[:, 0:1],
                                       x_sb[:, ts(ib, CH)], AOP.add, AOP.add)
        nc.gpsimd.dma_start(env["out_d"].ap()[:, ts(ib, CH)], stage[:])

    # any prologue steps not covered by PLACEMENT run at the rep's end
    if nxt is not None:
        for name, fn in nxt["steps"]:
            if name not in emitted:
                emitted.add(name)
                fn()


_NC_CACHE = {}


def _get_nc(reps=1):
    key = reps
    if key not in _NC_CACHE:
        _NC_CACHE[key] = _build_program(reps)
    return _NC_CACHE[key]


def _make_in_maps(x, gn_weight, gn_bias, qkv_weight, qkv_bias, proj_weight,
                  proj_bias):
    x = np.ascontiguousarray(x, dtype=np.float32)
    qkv_weight = np.asarray(qkv_weight, dtype=np.float32)
    qkv_bias = np.asarray(qkv_bias, dtype=np.float32)
    proj_weight = np.asarray(proj_weight, dtype=np.float32)
    proj_bias = np.asarray(proj_bias, dtype=np.float32)
    gn_weight = np.asarray(gn_weight, dtype=np.float32)
    gn_bias = np.asarray(gn_bias, dtype=np.float32)

    b = x.shape[0]
    xf = x.reshape(b, C, N)
    wqT = np.ascontiguousarray(qkv_weight[0:C].T)
    wkT = np.ascontiguousarray(qkv_weight[C:2 * C].T)
    wvT = np.ascontiguousarray(qkv_weight[2 * C:3 * C].T)
    wpT = np.ascontiguousarray(proj_weight.T)
    qkvb = np.ascontiguousarray(qkv_bias.reshape(3, C).T)
    pbv = np.ascontiguousarray(proj_bias.reshape(C, 1))
    gnwv = np.ascontiguousarray(gn_weight.reshape(C, 1))
    gnbv = np.ascontiguousarray(gn_bias.reshape(C, 1))

    in_maps = []
    for core in range(8):
        bi, half = core // 2, core % 2
        xc = xf[bi]
        if half == 1:  # own query half first; k/v order is irrelevant
            xc = np.concatenate([xc[:, NH:], xc[:, :NH]], axis=1)
        in_maps.append({
            "x": np.ascontiguousarray(xc),
            "wqT": wqT, "wkT": wkT, "wvT": wvT, "wpT": wpT,
            "qkvb": qkvb, "pb": pbv, "gnw": gnwv, "gnb": gnbv,
        })
    return in_maps


def run_on_cores(trace=False, reps=1, **inputs):
    """Build + run on the 8 cores; returns (BassKernelResults, output array)."""
    nc = _get_nc(reps)
    in_maps = _make_in_maps(**inputs)
    res = run_bass_kernel_spmd(nc, in_maps, core_ids=list(range(8)),
                               trace=trace)
    b = np.asarray(inputs["x"]).shape[0]
    h = w = 64
    out = np.empty((b, C, N), dtype=np.float32)
    for core in range(8):
        bi, half = core // 2, core % 2
        out[bi][:, half * NH:(half + 1) * NH] = res.results[core]["out"]
    return res, out.reshape(b, C, h, w)


def kernel(**inputs) -> np.ndarray:
    _, out = run_on_cores(trace=False, **inputs)
    return out


# revision 24
# speedup vs baseline: 1.3615x; 1.3615x over previous
# Trainium2 Bass kernel for nn_Attention_35433480192757
#
# reference computation (b=4, c=128, h=w=64, n=h*w=4096):
#   GroupNorm(8, c) -> 1x1 conv qkv -> full [n, n] attention per batch
#   -> 1x1 conv proj -> residual add
#
# Sharding: 8 cores = 4 batches x 2 query-row halves. Each core computes the
# full k/v for its batch (cheap: the qkv matmuls are tiny) and attention for
# its 2048 query rows. Host-side, each core's x is column-PERMUTED so that
# its own query half occupies columns 0:2048 -- attention is invariant to
# the j-enumeration order, and this keeps the SPMD program identical across
# cores with no separate xq input.
#
# Per-core strategy (fp8 + DoubleRow PV + two-engine softmax exp):
#   - x kept as [c=128 partitions, n] fp32; GroupNorm folded into the qkv
#     weights (xn = x*s_c + t_c per channel, computed on device; rsqrt via
#     a vector-engine bit-trick seed + Newton).
#   - q,k,v are produced as fp8e4. QK^T runs as plain fp8 contraction-128
#     matmuls. PV and the softmax-denominator (ones) matmuls consume P as
#     fp8 DoubleRow pairs.
#   - The k bias cancels in softmax; the v bias is folded into the proj
#     bias (pb' = pb + Wp @ bv).
#   - exp(scores) is written to fp8e5 by BOTH the scalar engine (true exp)
#     and the vector engine (Schraudolph bit-trick), split by a static
#     per-pair schedule.
#   - The whole per-rep prologue (GroupNorm stats, weight folds, k/q/v
#     production) is SOFTWARE-PIPELINED one rep ahead: its instructions are
#     emitted interleaved into the previous rep's attention pair loops, so
#     its PSUM tiles slot into the "sc"/"vv" rotations mid-stream and the
#     scalar engine never drains at rep boundaries.

import numpy as np
from contextlib import ExitStack

import concourse.bass as bass
from concourse import bacc
import concourse.tile as tile
import concourse.mybir as mybir
from concourse.bass import ts
from concourse.bass_utils import run_bass_kernel_spmd

P = 128          # partitions == channels
C = 128
N = 4096         # sequence length (h*w) per batch
NH = 2048        # query rows per core
CH = 512         # free-dim chunk (one PSUM bank of fp32)
NCH = N // CH    # 8 column chunks of x
NQCH = NH // CH  # 4 column chunks of q
NJC = N // P     # 32 key chunks (contraction over j)
NG = NJC // 2    # 16 j-chunk pairs per i-block
NIB = NH // CH   # 4 i-blocks per core
NUM_GROUPS = 8
GSIZE = C // NUM_GROUPS
EPS = 1e-5
SCALE = float(C) ** -0.5

F32 = mybir.dt.float32
F32R = mybir.dt.float32r
FP8 = mybir.dt.float8e4
FP8W = mybir.dt.float8e5   # P matrix: wide-range fp8 (e5m2)
I8 = mybir.dt.int8
AOP = mybir.AluOpType
AFT = mybir.ActivationFunctionType
DR = mybir.MatmulPerfMode.DoubleRow

# Schraudolph exp for fp8e5 (bias 15, 2 mantissa bits):
#   fp8e5_bits(exp(x)) ~= trunc(x * 4*log2e + 60 + c). e5m2's range covers
#   exp of +-10 sigma scores, so no clamping or shifting is needed; c=0.494
#   zeroes the mean multiplicative bias of the truncation.
SCHRAU_A = 4 * 1.4426950408889634
SCHRAU_B = 60.0 + 0.494

# Per-i-block sets of j-chunk pairs whose exp runs on the vector engine
# (Schraudolph); the rest use the scalar engine's exp.
DVE_EXP = {
    0: (1, 3, 5, 7, 9),
    1: (0, 2, 4, 6, 9, 11, 13),
    2: (1, 3, 5, 7, 9, 11, 13, 15),
    3: (2, 5, 8, 11, 14),
}

QK_PM = None
import os as _os
if _os.environ.get("QK_DP"):
    QK_PM = mybir.MatmulPerfMode.DoublePixel
if _os.environ.get("DVE_EXP_CFG"):
    # e.g. "1,3,5,7,9,11,13|2,4,6,8,11,14|1,4,6,9,11,13|2,5,8,11,14"
    _parts = _os.environ["DVE_EXP_CFG"].split("|")
    DVE_EXP = {i: tuple(int(v) for v in p.split(",") if v != "")
               for i, p in enumerate(_parts)}


def _build_program(reps=1):
    nc = bacc.Bacc(trn_type="TRN2", num_devices=8)

    x_d = nc.dram_tensor("x", [P, N], F32R, kind="ExternalInput")
    wqT_d = nc.dram_tensor("wqT", [P, P], F32, kind="ExternalInput")
    wkT_d = nc.dram_tensor("wkT", [P, P], F32, kind="ExternalInput")
    wvT_d = nc.dram_tensor("wvT", [P, P], F32, kind="ExternalInput")
    wpT_d = nc.dram_tensor("wpT", [P, P], F32R, kind="ExternalInput")
    qkvb_d = nc.dram_tensor("qkvb", [P, 3], F32, kind="ExternalInput")
    pb_d = nc.dram_tensor("pb", [P, 1], F32, kind="ExternalInput")
    gnw_d = nc.dram_tensor("gnw", [P, 1], F32, kind="ExternalInput")
    gnb_d = nc.dram_tensor("gnb", [P, 1], F32, kind="ExternalInput")
    out_d = nc.dram_tensor("out", [P, NH], F32, kind="ExternalOutput")

    gmat_np = np.zeros((P, P), dtype=np.float32)
    for g in range(NUM_GROUPS):
        gmat_np[g * GSIZE:(g + 1) * GSIZE, g * GSIZE:(g + 1) * GSIZE] = 1.0 / GSIZE
    gmat_d = nc.inline_tensor(gmat_np, "gmat")

    with ExitStack() as ctx:
        tc = ctx.enter_context(tile.TileContext(nc))

        consts = ctx.enter_context(tc.tile_pool(name="consts", bufs=1))
        wfold = ctx.enter_context(tc.tile_pool(name="wfold", bufs=2))
        xpool = ctx.enter_context(tc.tile_pool(name="xpool", bufs=2))
        kqv = ctx.enter_context(tc.tile_pool(name="kqv", bufs=2))
        ptp = ctx.enter_context(tc.tile_pool(name="ptp", bufs=3))
        work = ctx.enter_context(tc.tile_pool(name="work", bufs=2))
        small = ctx.enter_context(tc.tile_pool(name="small", bufs=2))
        outp = ctx.enter_context(tc.tile_pool(name="outp", bufs=2))
        psb = ctx.enter_context(tc.tile_pool(name="psb", bufs=3, space="PSUM"))
        psv = psb
        psacc = ctx.enter_context(tc.tile_pool(name="psacc", bufs=1, space="PSUM"))
        pssum = ctx.enter_context(tc.tile_pool(name="pssum", bufs=1, space="PSUM"))

        env = dict(
            nc=nc, consts=consts, wfold=wfold, xpool=xpool, kqv=kqv, ptp=ptp,
            work=work, small=small, outp=outp, psb=psb, psv=psv, psacc=psacc,
            pssum=pssum, x_d=x_d, wqT_d=wqT_d, wkT_d=wkT_d, wvT_d=wvT_d,
            wpT_d=wpT_d, qkvb_d=qkvb_d, pb_d=pb_d, gnw_d=gnw_d, gnb_d=gnb_d,
            gmat_d=gmat_d, out_d=out_d,
        )

        _emit_consts(env)

        # rep 0 prologue emitted serially upfront (cold start).
        st0 = _prologue_steps(env, 0)
        for _, fn in st0["steps"]:
            fn()
        states = [st0]

        for r in range(reps):
            nxt = _prologue_steps(env, r + 1) if r + 1 < reps else None
            _emit_attention(env, states[r], nxt)
            if nxt is not None:
                states.append(nxt)

    nc.compile()
    return nc


def _emit_consts(env):
    """One-time loads: weights, biases, gmat, ones; PE warmup."""
    nc = env["nc"]
    consts = env["consts"]

    wq = consts.tile([P, P], F32, tag="wq", name="wq")
    nc.sync.dma_start(wq[:], env["wqT_d"].ap())
    wk = consts.tile([P, P], F32, tag="wk", name="wk")
    nc.sync.dma_start(wk[:], env["wkT_d"].ap())
    wv = consts.tile([P, P], F32, tag="wv", name="wv")
    nc.sync.dma_start(wv[:], env["wvT_d"].ap())
    wp = consts.tile([P, P], F32R, tag="wp", name="wp")
    nc.sync.dma_start(wp[:], env["wpT_d"].ap())
    qkvb = consts.tile([P, 3], F32, tag="qkvb", name="qkvb")
    nc.sync.dma_start(qkvb[:], env["qkvb_d"].ap())
    pb = consts.tile([P, 1], F32, tag="pb", name="pb")
    nc.sync.dma_start(pb[:], env["pb_d"].ap())
    gnw = consts.tile([P, 1], F32, tag="gnw", name="gnw")
    nc.sync.dma_start(gnw[:], env["gnw_d"].ap())
    gnb = consts.tile([P, 1], F32, tag="gnb", name="gnb")
    nc.sync.dma_start(gnb[:], env["gnb_d"].ap())
    ones8 = consts.tile([P, 2, P], FP8, tag="ones8", name="ones8")
    nc.gpsimd.memset(ones8[:], 1.0)
    # gmat last on the queue: the warmup matmul below absorbs the DMA-queue
    # semaphore wait once (walrus codegen allows only one sync-wait on a
    # self-loading fp32 matmul).
    gmat = consts.tile([P, P], F32, tag="gmat", name="gmat")
    nc.sync.dma_start(gmat[:], env["gmat_d"].ap())

    ps_w = env["psb"].tile([P, 8], F32, tag="sc", name="ps_warm")
    nc.tensor.matmul(ps_w[:, 0:2], lhsT=gmat[:], rhs=gmat[:, 0:2])

    env["wq"], env["wk"], env["wv"], env["wp"] = wq, wk, wv, wp
    env["qkvb"], env["pb"], env["gnw"], env["gnb"] = qkvb, pb, gnw, gnb
    env["gmat"], env["ones8"] = gmat, ones8


def _prologue_steps(env, rep):
    """Build the prologue for rep `rep` as a state dict + ordered list of
    emission callbacks (to run serially for rep 0, or interleaved into the
    previous rep's attention loops)."""
    nc = env["nc"]
    small, wfold, xpool, kqv = (env["small"], env["wfold"], env["xpool"],
                                env["kqv"])
    psb, psv = env["psb"], env["psv"]

    st = {}
    steps = []

    def add(name):
        def deco(fn):
            steps.append((name, fn))
            return fn
        return deco

    @add("xdma")
    def _():
        x_sb = xpool.tile([P, N], F32R, tag="x", name=f"x_sb{rep}")
        st["x"] = x_sb
        for s in range(NCH // 2):
            eng = nc.sync if s % 2 == 0 else nc.gpsimd
            eng.dma_start(x_sb[:, ts(s, 2 * CH)], env["x_d"].ap()[:, ts(s, 2 * CH)])
        st["stats"] = small.tile([P, NCH // 2, 6], F32, tag="stats",
                                 name=f"stats{rep}")

    for _s in range(NCH // 2):
        def _stats(s=_s):
            nc.vector.bn_stats(st["stats"][:, s, :], st["x"][:, ts(s, 2 * CH)])
        steps.append((f"stats{_s}", _stats))

    @add("aggr")
    def _():
        mv = small.tile([P, 2], F32, tag="mv", name=f"mv{rep}")
        nc.vector.bn_aggr(mv[:], st["stats"][:])
        # t2 = [mean_c, E[x^2]_c]; prep on the (idle) gpsimd engine so the
        # chain does not queue behind DVE exps.
        t2 = small.tile([P, 2], F32, tag="t2", name=f"t2{rep}")
        nc.vector.tensor_copy(t2[:, 0:1], mv[:, 0:1])
        nc.vector.scalar_tensor_tensor(t2[:, 1:2], mv[:, 0:1], mv[:, 0:1],
                                       mv[:, 1:2], AOP.mult, AOP.add)
        st["t2"] = t2

    @add("gnmm")
    def _():
        # group stats via block-diagonal averaging matrix; the tiny PSUM
        # tile borrows a slot of the "sc" pair rotation.
        ps_t = psb.tile([P, 8], F32, tag="sc", name=f"ps_gn{rep}")
        nc.tensor.matmul(ps_t[:, 0:2], lhsT=env["gmat"][:], rhs=st["t2"][:])
        gstat = small.tile([P, 2], F32, tag="gstat", name=f"gstat{rep}")
        nc.scalar.copy(gstat[:], ps_t[:, 0:2])
        st["gstat"] = gstat

    @add("rstd")
    def _():
        # whole chain on DVE in one batch (walrus allows no ALU ops on Pool)
        gstat = st["gstat"]
        varn = small.tile([P, 1], F32, tag="varn", name=f"varn{rep}")
        nc.vector.scalar_tensor_tensor(varn[:], gstat[:, 0:1], gstat[:, 0:1],
                                       gstat[:, 1:2], AOP.mult, AOP.subtract)
        vpos = small.tile([P, 1], F32, tag="vpos", name=f"vpos{rep}")
        nc.vector.tensor_scalar(vpos[:], varn[:], -1.0, EPS, AOP.mult, AOP.add)
        tsh = small.tile([P, 1], mybir.dt.int32, tag="tsh", name=f"tsh{rep}")
        nc.vector.tensor_scalar(tsh[:], vpos[:].bitcast(mybir.dt.int32), 1,
                                None, AOP.arith_shift_right)
        rstd = small.tile([P, 1], F32, tag="rstd", name=f"rstd{rep}")
        nc.vector.tensor_scalar(rstd[:].bitcast(mybir.dt.int32), tsh[:], -1,
                                0x5f3759df, AOP.mult, AOP.add)
        for it in range(2):
            nt = small.tile([P, 1], F32, tag="nt", name=f"nt{rep}_{it}")
            nc.vector.tensor_mul(nt[:], rstd[:], rstd[:])
            nc.vector.tensor_mul(nt[:], nt[:], vpos[:])
            nc.vector.tensor_scalar(nt[:], nt[:], -0.5, 1.5, AOP.mult, AOP.add)
            nc.vector.tensor_mul(rstd[:], rstd[:], nt[:])
        s_c = small.tile([P, 1], F32, tag="s_c", name=f"s_c{rep}")
        nc.vector.tensor_mul(s_c[:], rstd[:], env["gnw"][:])
        # t_n = mean_g*s_c - gn_bias = -t_c
        t_n = small.tile([P, 1], F32, tag="t_n", name=f"t_n{rep}")
        nc.vector.scalar_tensor_tensor(t_n[:], gstat[:, 0:1], s_c[:],
                                       env["gnb"][:], AOP.mult, AOP.subtract)
        st["s_c"], st["t_n"] = s_c, t_n

    @add("foldq")
    def _():
        wq_s = wfold.tile([P, P], F32R, tag="wq_s", name=f"wq_s{rep}")
        nc.vector.tensor_scalar_mul(wq_s[:], env["wq"][:], st["s_c"][:])
        st["wq_s"] = wq_s

    @add("foldk")
    def _():
        wk_s = wfold.tile([P, P], F32R, tag="wk_s", name=f"wk_s{rep}")
        nc.vector.tensor_scalar_mul(wk_s[:], env["wk"][:], st["s_c"][:])
        st["wk_s"] = wk_s

    @add("foldv")
    def _():
        wv_s = wfold.tile([P, 2, P], F32R, tag="wv_s", name=f"wv_s{rep}")
        nc.vector.tensor_scalar_mul(wv_s[:, 0, :], env["wv"][:], st["s_c"][:])
        nc.vector.tensor_scalar_mul(wv_s[:, 1, :], env["wv"][:], st["s_c"][:])
        st["wv_s"] = wv_s

    @add("bias")
    def _():
        # q bias (k bias cancels; v bias folds into the proj bias below)
        ps_b = psb.tile([P, 8], F32, tag="sc", name=f"ps_b{rep}")
        nc.tensor.matmul(ps_b[:, 0:1], lhsT=env["wq"][:], rhs=st["t_n"][:])
        nc.tensor.matmul(ps_b[:, 1:2], lhsT=env["wv"][:], rhs=st["t_n"][:])
        bq = small.tile([P, 1], F32, tag="bq", name=f"bq{rep}")
        nc.scalar.activation(bq[:], ps_b[:, 0:1], AFT.Identity,
                             bias=env["qkvb"][:, 0:1], scale=-1.0)
        bv = small.tile([P, 1], F32, tag="bv", name=f"bv{rep}")
        nc.scalar.activation(bv[:], ps_b[:, 1:2], AFT.Identity,
                             bias=env["qkvb"][:, 2:3], scale=-1.0)
        st["bq"], st["bv"] = bq, bv

    @add("pbf")
    def _():
        ps_p = psb.tile([P, 8], F32, tag="sc", name=f"ps_p{rep}")
        nc.tensor.matmul(ps_p[:, 0:1], lhsT=env["wp"][:].bitcast(F32),
                         rhs=st["bv"][:])
        pbf = small.tile([P, 1], F32, tag="pbf", name=f"pbf{rep}")
        nc.scalar.activation(pbf[:], ps_p[:, 0:1], AFT.Identity,
                             bias=env["pb"][:], scale=1.0)
        st["pbf"] = pbf
        st["kT8"] = kqv.tile([P, N], FP8, tag="kdr", name=f"kT8{rep}")
        st["qT8"] = kqv.tile([P, NH], FP8, tag="qdr", name=f"qT8{rep}")
        st["vnat"] = kqv.tile([P, NJC, P], FP8, tag="vnat", name=f"vnat{rep}")

    for _s in range(NCH // 2):
        def _k(s=_s):
            # two 512-col matmuls into one 2-bank tile; ONE 1024-col copy
            pk = psb.tile([P, 2, CH], F32, tag="sc", name=f"psk{rep}_{s}")
            for h in range(2):
                nc.tensor.matmul(pk[:, h, :], lhsT=st["wk_s"][:],
                                 rhs=st["x"][:, ts(2 * s + h, CH)])
            nc.scalar.copy(st["kT8"][:, ts(s, 2 * CH)], pk[:])
        steps.append((f"k{_s}", _k))

    for _s in range(NQCH // 2):
        def _q(s=_s):
            # q carries the bias: scalar-engine activation applies it
            pq = psb.tile([P, 2, CH], F32, tag="sc", name=f"psq{rep}_{s}")
            for h in range(2):
                nc.tensor.matmul(pq[:, h, :], lhsT=st["wq_s"][:],
                                 rhs=st["x"][:, ts(2 * s + h, CH)])
            nc.scalar.activation(st["qT8"][:, ts(s, 2 * CH)], pq[:],
                                 AFT.Identity, bias=st["bq"][:], scale=1.0)
        steps.append((f"q{_s}", _q))

    for _g in range(NJC // 4):
        def _v(q8=_g):
            # four chunks per PSUM tile, each duplicated twice (free=256
            # keeps fp32r at full rate); one copy reads the first replicas.
            pv = psv.tile([P, 4, 2, P], F32, tag="sc", name=f"psv{rep}_{q8}")
            for h in range(4):
                jc = 4 * q8 + h
                nc.tensor.matmul(pv[:, h, :, :],
                                 lhsT=st["x"][:, jc * P:(jc + 1) * P],
                                 rhs=st["wv_s"][:])
            nc.vector.tensor_copy(st["vnat"][:, 4 * q8:4 * q8 + 4, :],
                                  pv[:, :, 0, :])
        steps.append((f"v{_g}", _v))

    st["steps"] = steps
    return st


# Placement of next-rep prologue steps inside the current rep's attention:
# {(ib, pair_g): [step names]}. Steps not listed here run at their default
# position (appended after the pair loop of the listed block).
def _placement():
    pl = {}
    pl[(0, None)] = ["xdma"]                     # x DMA at ib0 entry
    # stats: 4 double-chunks over ib0's back half (no DVE exps there)
    for i in range(NCH // 2):
        pl[(0, 8 + 2 * i)] = [f"stats{i}"]
    pl[(1, 1)] = ["aggr"]
    pl[(1, 3)] = ["gnmm"]
    pl[(1, 5)] = ["rstd"]
    pl[(1, 8)] = ["foldq", "foldk"]
    pl[(1, 10)] = ["foldv"]
    pl[(1, 12)] = ["bias"]
    pl[(1, 14)] = ["pbf"]
    for i in range(NCH // 2):                     # k production over ib2
        pl[(2, 4 + 2 * i)] = [f"k{i}"]
    for i in range(NQCH // 2):                    # q production at ib2 end
        pl[(2, 12 + 2 * i)] = [f"q{i}"]
    # v production: 8 calls over ib3 pairs 0..14 (one every other pair)
    for i in range(NJC // 4):
        pl.setdefault((3, 2 * i), []).append(f"v{i}")
    return pl


PLACEMENT = _placement()


def _emit_attention(env, st, nxt):
    nc = env["nc"]
    ptp, work, outp = env["ptp"], env["work"], env["outp"]
    psb, psv, psacc, pssum = env["psb"], env["psv"], env["psacc"], env["pssum"]

    nxt_steps = dict(nxt["steps"]) if nxt is not None else {}
    emitted = set()

    def run_extra(ib, g):
        for name in PLACEMENT.get((ib, g), []) or []:
            fn = nxt_steps.get(name)
            if fn is not None and name not in emitted:
                emitted.add(name)
                fn()

    x_sb, kT8, qT8, vnat = st["x"], st["kT8"], st["qT8"], st["vnat"]
    wp, pbf = env["wp"], st["pbf"]

    for ib in range(NIB):
        if nxt is not None:
            run_extra(ib, None)
        PT = ptp.tile([P, NJC, CH], FP8W, tag="PT", name=f"PT{ib}")
        acc = psacc.tile([P, CH], F32, tag="acc", name=f"acc{ib}")
        sm = pssum.tile([P, CH], F32, tag="sp", name=f"sm{ib}")
        qblk = qT8[:, ts(ib, CH)]
        dve_pairs = DVE_EXP[ib]

        def emit_pv(g):
            pair = PT[:, 2 * g:2 * g + 2, :]
            nc.tensor.matmul(acc[:], lhsT=vnat[:, 2 * g:2 * g + 2, :],
                             rhs=pair, perf_mode=DR,
                             start=(g == 0), stop=(g == NG - 1),
                             skip_group_check=True)
            nc.tensor.matmul(sm[:], lhsT=env["ones8"][:], rhs=pair,
                             perf_mode=DR, start=(g == 0), stop=(g == NG - 1),
                             skip_group_check=True)

        for g in range(NG):
            dve_own = g in dve_pairs
            ps = psb.tile([P, 2, CH], F32, tag="sc", name=f"ps{ib}_{g}")
            for h in range(2):
                jc = 2 * g + h
                kslice = kT8[:, jc * P:(jc + 1) * P]
                nc.tensor.matmul(ps[:, h, :], lhsT=kslice, rhs=qblk,
                                 perf_mode=QK_PM, skip_group_check=True)
            if dve_own:
                nc.vector.tensor_scalar(PT[:, 2 * g:2 * g + 2, :].bitcast(I8),
                                        ps[:], SCHRAU_A * SCALE, SCHRAU_B,
                                        AOP.mult, AOP.add)
            run_extra(ib, g)
            if g > 3:
                emit_pv(g - 4)
            if not dve_own:
                nc.scalar.activation(PT[:, 2 * g:2 * g + 2, :], ps[:],
                                     AFT.Exp, scale=SCALE)
        emit_pv(NG - 4)
        emit_pv(NG - 3)
        emit_pv(NG - 2)
        emit_pv(NG - 1)

        # normalize and project
        recip = work.tile([P, CH], F32, tag="recip", name=f"recip{ib}")
        nc.vector.reciprocal_approx_fast(recip[:], sm[:])
        outn = work.tile([P, CH], F32R, tag="outn", name=f"outn{ib}")
        nc.vector.tensor_mul(outn[:], acc[:], recip[:])

        psp = pssum.tile([P, CH], F32, tag="sp", name=f"psp{ib}")
        nc.tensor.matmul(psp[:], lhsT=wp[:], rhs=outn[:])
        stage = outp.tile([P, CH], F32, tag="stage", name=f"stage{ib}")
        nc.vector.scalar_tensor_tensor(stage[:], psp[:], pbf[:, 0:1],
                                       x_sb[:, ts(ib, CH)], AOP.add, AOP.add)
        nc.gpsimd.dma_start(env["out_d"].ap()[:, ts(ib, CH)], stage[:])

    # any prologue steps not covered by PLACEMENT run at the rep's end
    if nxt is not None:
        for name, fn in nxt["steps"]:
            if name not in emitted:
                emitted.add(name)
                fn()


_NC_CACHE = {}


def _get_nc(reps=1):
    key = reps
    if key not in _NC_CACHE:
        _NC_CACHE[key] = _build_program(reps)
    return _NC_CACHE[key]


def _make_in_maps(x, gn_weight, gn_bias, qkv_weight, qkv_bias, proj_weight,
                  proj_bias):
    x = np.ascontiguousarray(x, dtype=np.float32)
    qkv_weight = np.asarray(qkv_weight, dtype=np.float32)
    qkv_bias = np.asarray(qkv_bias, dtype=np.float32)
    proj_weight = np.asarray(proj_weight, dtype=np.float32)
    proj_bias = np.asarray(proj_bias, dtype=np.float32)
    gn_weight = np.asarray(gn_weight, dtype=np.float32)
    gn_bias = np.asarray(gn_bias, dtype=np.float32)

    b = x.shape[0]
    xf = x.reshape(b, C, N)
    wqT = np.ascontiguousarray(qkv_weight[0:C].T)
    wkT = np.ascontiguousarray(qkv_weight[C:2 * C].T)
    wvT = np.ascontiguousarray(qkv_weight[2 * C:3 * C].T)
    wpT = np.ascontiguousarray(proj_weight.T)
    qkvb = np.ascontiguousarray(qkv_bias.reshape(3, C).T)
    pbv = np.ascontiguousarray(proj_bias.reshape(C, 1))
    gnwv = np.ascontiguousarray(gn_weight.reshape(C, 1))
    gnbv = np.ascontiguousarray(gn_bias.reshape(C, 1))

    in_maps = []
    for core in range(8):
        bi, half = core // 2, core % 2
        xc = xf[bi]
        if half == 1:  # own query half first; k/v order is irrelevant
            xc = np.concatenate([xc[:, NH:], xc[:, :NH]], axis=1)
        in_maps.append({
            "x": np.ascontiguousarray(xc),
            "wqT": wqT, "wkT": wkT, "wvT": wvT, "wpT": wpT,
            "qkvb": qkvb, "pb": pbv, "gnw": gnwv, "gnb": gnbv,
        })
    return in_maps


def run_on_cores(trace=False, reps=1, **inputs):
    """Build + run on the 8 cores; returns (BassKernelResults, output array)."""
    nc = _get_nc(reps)
    in_maps = _make_in_maps(**inputs)
    res = run_bass_kernel_spmd(nc, in_maps, core_ids=list(range(8)),
                               trace=trace)
    b = np.asarray(inputs["x"]).shape[0]
    h = w = 64
    out = np.empty((b, C, N), dtype=np.float32)
    for core in range(8):
        bi, half = core // 2, core % 2
        out[bi][:, half * NH:(half + 1) * NH] = res.results[core]["out"]
    return res, out.reshape(b, C, h, w)


def kernel(**inputs) -> np.ndarray:
    _, out = run_on_cores(trace=False, **inputs)
    return out


# revision 25
# speedup vs baseline: 1.4650x; 1.0760x over previous
# Trainium2 Bass kernel for nn_Attention_35433480192757
#
# reference computation (b=4, c=128, h=w=64, n=h*w=4096):
#   GroupNorm(8, c) -> 1x1 conv qkv -> full [n, n] attention per batch
#   -> 1x1 conv proj -> residual add
#
# Sharding: 8 cores = 4 batches x 2 query-row halves. Each core computes the
# full k/v for its batch (cheap: the qkv matmuls are tiny) and attention for
# its 2048 query rows. Host-side, each core's x is column-PERMUTED so that
# its own query half occupies columns 0:2048 -- attention is invariant to
# the j-enumeration order, and this keeps the SPMD program identical across
# cores with no separate xq input.
#
# Per-core strategy (fp8 + DoubleRow PV + two-engine softmax exp):
#   - x kept as [c=128 partitions, n] fp32; GroupNorm folded into the qkv
#     weights (xn = x*s_c + t_c per channel, computed on device; rsqrt via
#     a vector-engine bit-trick seed + Newton).
#   - q,k,v are produced as fp8e4. QK^T runs as plain fp8 contraction-128
#     matmuls. PV and the softmax-denominator (ones) matmuls consume P as
#     fp8 DoubleRow pairs.
#   - The k bias cancels in softmax; the v bias is folded into the proj
#     bias (pb' = pb + Wp @ bv).
#   - exp(scores) is written to fp8e5 by BOTH the scalar engine (true exp)
#     and the vector engine (Schraudolph bit-trick), split by a static
#     per-pair schedule.
#   - The whole per-rep prologue (GroupNorm stats, weight folds, k/q/v
#     production) is SOFTWARE-PIPELINED one rep ahead: its instructions are
#     emitted interleaved into the previous rep's attention pair loops, so
#     its PSUM tiles slot into the "sc"/"vv" rotations mid-stream and the
#     scalar engine never drains at rep boundaries.

import numpy as np
from contextlib import ExitStack

import concourse.bass as bass
from concourse import bacc
import concourse.tile as tile
import concourse.mybir as mybir
from concourse.bass import ts
from concourse.bass_utils import run_bass_kernel_spmd

P = 128          # partitions == channels
C = 128
N = 4096         # sequence length (h*w) per batch
NH = 2048        # query rows per core
CH = 512         # free-dim chunk (one PSUM bank of fp32)
NCH = N // CH    # 8 column chunks of x
NQCH = NH // CH  # 4 column chunks of q
NJC = N // P     # 32 key chunks (contraction over j)
NG = NJC // 2    # 16 j-chunk pairs per i-block
NIB = NH // CH   # 4 i-blocks per core
NUM_GROUPS = 8
GSIZE = C // NUM_GROUPS
EPS = 1e-5
SCALE = float(C) ** -0.5

F32 = mybir.dt.float32
F32R = mybir.dt.float32r
FP8 = mybir.dt.float8e4
FP8W = mybir.dt.float8e5   # P matrix: wide-range fp8 (e5m2)
I8 = mybir.dt.int8
AOP = mybir.AluOpType
AFT = mybir.ActivationFunctionType
DR = mybir.MatmulPerfMode.DoubleRow

# Schraudolph exp for fp8e5 (bias 15, 2 mantissa bits):
#   fp8e5_bits(exp(x)) ~= trunc(x * 4*log2e + 60 + c). e5m2's range covers
#   exp of +-10 sigma scores, so no clamping or shifting is needed; c=0.494
#   zeroes the mean multiplicative bias of the truncation.
SCHRAU_A = 4 * 1.4426950408889634
SCHRAU_B = 60.0 + 0.494

# Per-i-block sets of j-chunk pairs whose exp runs on the vector engine
# (Schraudolph); the rest use the scalar engine's exp.
DVE_EXP = {
    0: (1, 3, 5, 7, 9),
    1: (0, 2, 4, 6, 9, 11, 13),
    2: (1, 3, 5, 7, 9, 11, 13, 15),
    3: (2, 5, 8, 11, 14),
}

QK_PM = None
import os as _os
if _os.environ.get("QK_DP"):
    QK_PM = mybir.MatmulPerfMode.DoublePixel
if _os.environ.get("DVE_EXP_CFG"):
    # e.g. "1,3,5,7,9,11,13|2,4,6,8,11,14|1,4,6,9,11,13|2,5,8,11,14"
    _parts = _os.environ["DVE_EXP_CFG"].split("|")
    DVE_EXP = {i: tuple(int(v) for v in p.split(",") if v != "")
               for i, p in enumerate(_parts)}


def _build_program(reps=1):
    nc = bacc.Bacc(trn_type="TRN2", num_devices=8)

    x_d = nc.dram_tensor("x", [P, N], F32R, kind="ExternalInput")
    wqT_d = nc.dram_tensor("wqT", [P, P], F32, kind="ExternalInput")
    wkT_d = nc.dram_tensor("wkT", [P, P], F32, kind="ExternalInput")
    wvT_d = nc.dram_tensor("wvT", [P, P], F32, kind="ExternalInput")
    wpT_d = nc.dram_tensor("wpT", [P, P], F32R, kind="ExternalInput")
    qkvb_d = nc.dram_tensor("qkvb", [P, 3], F32, kind="ExternalInput")
    pb_d = nc.dram_tensor("pb", [P, 1], F32, kind="ExternalInput")
    gnw_d = nc.dram_tensor("gnw", [P, 1], F32, kind="ExternalInput")
    gnb_d = nc.dram_tensor("gnb", [P, 1], F32, kind="ExternalInput")
    out_d = nc.dram_tensor("out", [P, NH], F32, kind="ExternalOutput")

    gmat_np = np.zeros((P, P), dtype=np.float32)
    for g in range(NUM_GROUPS):
        gmat_np[g * GSIZE:(g + 1) * GSIZE, g * GSIZE:(g + 1) * GSIZE] = 1.0 / GSIZE
    gmat_d = nc.inline_tensor(gmat_np, "gmat")

    with ExitStack() as ctx:
        tc = ctx.enter_context(tile.TileContext(nc))

        consts = ctx.enter_context(tc.tile_pool(name="consts", bufs=1))
        wfold = ctx.enter_context(tc.tile_pool(name="wfold", bufs=2))
        xpool = ctx.enter_context(tc.tile_pool(name="xpool", bufs=2))
        kqv = ctx.enter_context(tc.tile_pool(name="kqv", bufs=2))
        ptp = ctx.enter_context(tc.tile_pool(name="ptp", bufs=3))
        work = ctx.enter_context(tc.tile_pool(name="work", bufs=2))
        small = ctx.enter_context(tc.tile_pool(name="small", bufs=2))
        outp = ctx.enter_context(tc.tile_pool(name="outp", bufs=2))
        psb = ctx.enter_context(tc.tile_pool(name="psb", bufs=3, space="PSUM"))
        psv = psb
        psacc = ctx.enter_context(tc.tile_pool(name="psacc", bufs=1, space="PSUM"))
        pssum = ctx.enter_context(tc.tile_pool(name="pssum", bufs=1, space="PSUM"))

        env = dict(
            nc=nc, consts=consts, wfold=wfold, xpool=xpool, kqv=kqv, ptp=ptp,
            work=work, small=small, outp=outp, psb=psb, psv=psv, psacc=psacc,
            pssum=pssum, x_d=x_d, wqT_d=wqT_d, wkT_d=wkT_d, wvT_d=wvT_d,
            wpT_d=wpT_d, qkvb_d=qkvb_d, pb_d=pb_d, gnw_d=gnw_d, gnb_d=gnb_d,
            gmat_d=gmat_d, out_d=out_d,
        )

        _emit_consts(env)

        # rep 0 prologue emitted serially upfront (cold start).
        st0 = _prologue_steps(env, 0)
        for _, fn in st0["steps"]:
            fn()
        states = [st0]

        for r in range(reps):
            nxt = _prologue_steps(env, r + 1) if r + 1 < reps else None
            _emit_attention(env, states[r], nxt)
            if nxt is not None:
                states.append(nxt)

    nc.compile()
    return nc


def _emit_consts(env):
    """One-time loads: weights, biases, gmat, ones; PE warmup."""
    nc = env["nc"]
    consts = env["consts"]

    wq = consts.tile([P, P], F32, tag="wq", name="wq")
    nc.sync.dma_start(wq[:], env["wqT_d"].ap())
    wk = consts.tile([P, P], F32, tag="wk", name="wk")
    nc.sync.dma_start(wk[:], env["wkT_d"].ap())
    wv = consts.tile([P, P], F32, tag="wv", name="wv")
    nc.sync.dma_start(wv[:], env["wvT_d"].ap())
    wp = consts.tile([P, P], F32R, tag="wp", name="wp")
    nc.sync.dma_start(wp[:], env["wpT_d"].ap())
    qkvb = consts.tile([P, 3], F32, tag="qkvb", name="qkvb")
    nc.sync.dma_start(qkvb[:], env["qkvb_d"].ap())
    pb = consts.tile([P, 1], F32, tag="pb", name="pb")
    nc.sync.dma_start(pb[:], env["pb_d"].ap())
    gnw = consts.tile([P, 1], F32, tag="gnw", name="gnw")
    nc.sync.dma_start(gnw[:], env["gnw_d"].ap())
    gnb = consts.tile([P, 1], F32, tag="gnb", name="gnb")
    nc.sync.dma_start(gnb[:], env["gnb_d"].ap())
    ones8 = consts.tile([P, 2, P], FP8, tag="ones8", name="ones8")
    nc.gpsimd.memset(ones8[:], 1.0)
    # gmat last on the queue: the warmup matmul below absorbs the DMA-queue
    # semaphore wait once (walrus codegen allows only one sync-wait on a
    # self-loading fp32 matmul).
    gmat = consts.tile([P, P], F32, tag="gmat", name="gmat")
    nc.sync.dma_start(gmat[:], env["gmat_d"].ap())

    ps_w = env["psb"].tile([P, 8], F32, tag="sc", name="ps_warm")
    nc.tensor.matmul(ps_w[:, 0:2], lhsT=gmat[:], rhs=gmat[:, 0:2])

    env["wq"], env["wk"], env["wv"], env["wp"] = wq, wk, wv, wp
    env["qkvb"], env["pb"], env["gnw"], env["gnb"] = qkvb, pb, gnw, gnb
    env["gmat"], env["ones8"] = gmat, ones8


def _prologue_steps(env, rep):
    """Build the prologue for rep `rep` as a state dict + ordered list of
    emission callbacks (to run serially for rep 0, or interleaved into the
    previous rep's attention loops)."""
    nc = env["nc"]
    small, wfold, xpool, kqv = (env["small"], env["wfold"], env["xpool"],
                                env["kqv"])
    psb, psv = env["psb"], env["psv"]

    st = {}
    steps = []

    def add(name):
        def deco(fn):
            steps.append((name, fn))
            return fn
        return deco

    @add("xdma")
    def _():
        x_sb = xpool.tile([P, N], F32R, tag="x", name=f"x_sb{rep}")
        st["x"] = x_sb
        for s in range(NCH // 2):
            eng = nc.sync if s % 2 == 0 else nc.gpsimd
            eng.dma_start(x_sb[:, ts(s, 2 * CH)], env["x_d"].ap()[:, ts(s, 2 * CH)])
        st["stats"] = small.tile([P, NCH, 6], F32, tag="stats",
                                 name=f"stats{rep}")

    for _s in range(NCH):
        def _stats(s=_s):
            nc.vector.bn_stats(st["stats"][:, s, :], st["x"][:, ts(s, CH)])
        steps.append((f"stats{_s}", _stats))

    @add("aggr")
    def _():
        mv = small.tile([P, 2], F32, tag="mv", name=f"mv{rep}")
        nc.vector.bn_aggr(mv[:], st["stats"][:])
        # t2 = [mean_c, E[x^2]_c]; prep on the (idle) gpsimd engine so the
        # chain does not queue behind DVE exps.
        t2 = small.tile([P, 2], F32, tag="t2", name=f"t2{rep}")
        nc.vector.tensor_copy(t2[:, 0:1], mv[:, 0:1])
        nc.vector.scalar_tensor_tensor(t2[:, 1:2], mv[:, 0:1], mv[:, 0:1],
                                       mv[:, 1:2], AOP.mult, AOP.add)
        st["t2"] = t2

    @add("gnmm")
    def _():
        # group stats via block-diagonal averaging matrix; the tiny PSUM
        # tile borrows a slot of the "sc" pair rotation.
        ps_t = psb.tile([P, 8], F32, tag="sc", name=f"ps_gn{rep}")
        nc.tensor.matmul(ps_t[:, 0:2], lhsT=env["gmat"][:], rhs=st["t2"][:])
        gstat = small.tile([P, 2], F32, tag="gstat", name=f"gstat{rep}")
        nc.scalar.copy(gstat[:], ps_t[:, 0:2])
        st["gstat"] = gstat

    @add("rstd")
    def _():
        # whole chain on DVE in one batch (walrus allows no ALU ops on Pool)
        gstat = st["gstat"]
        varn = small.tile([P, 1], F32, tag="varn", name=f"varn{rep}")
        nc.vector.scalar_tensor_tensor(varn[:], gstat[:, 0:1], gstat[:, 0:1],
                                       gstat[:, 1:2], AOP.mult, AOP.subtract)
        vpos = small.tile([P, 1], F32, tag="vpos", name=f"vpos{rep}")
        nc.vector.tensor_scalar(vpos[:], varn[:], -1.0, EPS, AOP.mult, AOP.add)
        tsh = small.tile([P, 1], mybir.dt.int32, tag="tsh", name=f"tsh{rep}")
        nc.vector.tensor_scalar(tsh[:], vpos[:].bitcast(mybir.dt.int32), 1,
                                None, AOP.arith_shift_right)
        rstd = small.tile([P, 1], F32, tag="rstd", name=f"rstd{rep}")
        nc.vector.tensor_scalar(rstd[:].bitcast(mybir.dt.int32), tsh[:], -1,
                                0x5f3759df, AOP.mult, AOP.add)
        for it in range(2):
            nt = small.tile([P, 1], F32, tag="nt", name=f"nt{rep}_{it}")
            nc.vector.tensor_mul(nt[:], rstd[:], rstd[:])
            nc.vector.tensor_mul(nt[:], nt[:], vpos[:])
            nc.vector.tensor_scalar(nt[:], nt[:], -0.5, 1.5, AOP.mult, AOP.add)
            nc.vector.tensor_mul(rstd[:], rstd[:], nt[:])
        s_c = small.tile([P, 1], F32, tag="s_c", name=f"s_c{rep}")
        nc.vector.tensor_mul(s_c[:], rstd[:], env["gnw"][:])
        # t_n = mean_g*s_c - gn_bias = -t_c
        t_n = small.tile([P, 1], F32, tag="t_n", name=f"t_n{rep}")
        nc.vector.scalar_tensor_tensor(t_n[:], gstat[:, 0:1], s_c[:],
                                       env["gnb"][:], AOP.mult, AOP.subtract)
        st["s_c"], st["t_n"] = s_c, t_n

    @add("foldq")
    def _():
        wq_s = wfold.tile([P, P], F32R, tag="wq_s", name=f"wq_s{rep}")
        nc.vector.tensor_scalar_mul(wq_s[:], env["wq"][:], st["s_c"][:])
        st["wq_s"] = wq_s

    @add("foldk")
    def _():
        wk_s = wfold.tile([P, P], F32R, tag="wk_s", name=f"wk_s{rep}")
        nc.vector.tensor_scalar_mul(wk_s[:], env["wk"][:], st["s_c"][:])
        st["wk_s"] = wk_s

    @add("foldv")
    def _():
        wv_s = wfold.tile([P, 2, P], F32R, tag="wv_s", name=f"wv_s{rep}")
        nc.vector.tensor_scalar_mul(wv_s[:, 0, :], env["wv"][:], st["s_c"][:])
        nc.vector.tensor_scalar_mul(wv_s[:, 1, :], env["wv"][:], st["s_c"][:])
        st["wv_s"] = wv_s

    @add("bias")
    def _():
        # q bias (k bias cancels; v bias folds into the proj bias below)
        ps_b = psb.tile([P, 8], F32, tag="sc", name=f"ps_b{rep}")
        nc.tensor.matmul(ps_b[:, 0:1], lhsT=env["wq"][:], rhs=st["t_n"][:])
        nc.tensor.matmul(ps_b[:, 1:2], lhsT=env["wv"][:], rhs=st["t_n"][:])
        bq = small.tile([P, 1], F32, tag="bq", name=f"bq{rep}")
        nc.scalar.activation(bq[:], ps_b[:, 0:1], AFT.Identity,
                             bias=env["qkvb"][:, 0:1], scale=-1.0)
        bv = small.tile([P, 1], F32, tag="bv", name=f"bv{rep}")
        nc.scalar.activation(bv[:], ps_b[:, 1:2], AFT.Identity,
                             bias=env["qkvb"][:, 2:3], scale=-1.0)
        st["bq"], st["bv"] = bq, bv

    @add("pbf")
    def _():
        ps_p = psb.tile([P, 8], F32, tag="sc", name=f"ps_p{rep}")
        nc.tensor.matmul(ps_p[:, 0:1], lhsT=env["wp"][:].bitcast(F32),
                         rhs=st["bv"][:])
        pbf = small.tile([P, 1], F32, tag="pbf", name=f"pbf{rep}")
        nc.scalar.activation(pbf[:], ps_p[:, 0:1], AFT.Identity,
                             bias=env["pb"][:], scale=1.0)
        st["pbf"] = pbf
        st["kT8"] = kqv.tile([P, N], FP8, tag="kdr", name=f"kT8{rep}")
        st["qT8"] = kqv.tile([P, NH], FP8, tag="qdr", name=f"qT8{rep}")
        st["vnat"] = kqv.tile([P, NJC, P], FP8, tag="vnat", name=f"vnat{rep}")

    for _s in range(NCH // 2):
        def _k(s=_s):
            # two 512-col matmuls into one 2-bank tile; ONE 1024-col copy
            pk = psb.tile([P, 2, CH], F32, tag="sc", name=f"psk{rep}_{s}")
            for h in range(2):
                nc.tensor.matmul(pk[:, h, :], lhsT=st["wk_s"][:],
                                 rhs=st["x"][:, ts(2 * s + h, CH)])
            nc.scalar.copy(st["kT8"][:, ts(s, 2 * CH)], pk[:])
        steps.append((f"k{_s}", _k))

    for _s in range(NQCH // 2):
        def _q(s=_s):
            # q carries the bias: scalar-engine activation applies it
            pq = psb.tile([P, 2, CH], F32, tag="sc", name=f"psq{rep}_{s}")
            for h in range(2):
                nc.tensor.matmul(pq[:, h, :], lhsT=st["wq_s"][:],
                                 rhs=st["x"][:, ts(2 * s + h, CH)])
            nc.scalar.activation(st["qT8"][:, ts(s, 2 * CH)], pq[:],
                                 AFT.Identity, bias=st["bq"][:], scale=1.0)
        steps.append((f"q{_s}", _q))

    for _g in range(NJC // 4):
        def _v(q8=_g):
            # four chunks per PSUM tile, each duplicated twice (free=256
            # keeps fp32r at full rate); one copy reads the first replicas.
            pv = psv.tile([P, 4, 2, P], F32, tag="sc", name=f"psv{rep}_{q8}")
            for h in range(4):
                jc = 4 * q8 + h
                nc.tensor.matmul(pv[:, h, :, :],
                                 lhsT=st["x"][:, jc * P:(jc + 1) * P],
                                 rhs=st["wv_s"][:])
            nc.vector.tensor_copy(st["vnat"][:, 4 * q8:4 * q8 + 4, :],
                                  pv[:, :, 0, :])
        steps.append((f"v{_g}", _v))

    st["steps"] = steps
    return st


# Placement of next-rep prologue steps inside the current rep's attention:
# {(ib, pair_g): [step names]}. Steps not listed here run at their default
# position (appended after the pair loop of the listed block).
def _placement():
    pl = {}
    pl[(0, None)] = ["xdma"]                     # x DMA at ib0 entry
    # stats: all 8 chunks over ib0's back half (no DVE exps there)
    for i in range(NCH):
        pl[(0, 8 + i)] = [f"stats{i}"]
    pl[(1, 1)] = ["aggr"]
    pl[(1, 3)] = ["gnmm"]
    pl[(1, 5)] = ["rstd"]
    pl[(1, 8)] = ["foldq", "foldk"]
    pl[(1, 10)] = ["foldv"]
    pl[(1, 12)] = ["bias"]
    pl[(1, 14)] = ["pbf"]
    for i in range(NCH // 2):                     # k production over ib2
        pl[(2, 4 + 2 * i)] = [f"k{i}"]
    for i in range(NQCH // 2):                    # q production at ib2 end
        pl[(2, 12 + 2 * i)] = [f"q{i}"]
    # v production: 8 calls over ib3 pairs 0..14 (one every other pair)
    for i in range(NJC // 4):
        pl.setdefault((3, 2 * i), []).append(f"v{i}")
    return pl


PLACEMENT = _placement()


def _emit_attention(env, st, nxt):
    nc = env["nc"]
    ptp, work, outp = env["ptp"], env["work"], env["outp"]
    psb, psv, psacc, pssum = env["psb"], env["psv"], env["psacc"], env["pssum"]

    nxt_steps = dict(nxt["steps"]) if nxt is not None else {}
    emitted = set()

    def run_extra(ib, g):
        for name in PLACEMENT.get((ib, g), []) or []:
            fn = nxt_steps.get(name)
            if fn is not None and name not in emitted:
                emitted.add(name)
                fn()

    x_sb, kT8, qT8, vnat = st["x"], st["kT8"], st["qT8"], st["vnat"]
    wp, pbf = env["wp"], st["pbf"]

    for ib in range(NIB):
        if nxt is not None:
            run_extra(ib, None)
        PT = ptp.tile([P, NJC, CH], FP8W, tag="PT", name=f"PT{ib}")
        acc = psacc.tile([P, CH], F32, tag="acc", name=f"acc{ib}")
        sm = pssum.tile([P, CH], F32, tag="sp", name=f"sm{ib}")
        qblk = qT8[:, ts(ib, CH)]
        dve_pairs = DVE_EXP[ib]

        def emit_pv(g):
            pair = PT[:, 2 * g:2 * g + 2, :]
            nc.tensor.matmul(acc[:], lhsT=vnat[:, 2 * g:2 * g + 2, :],
                             rhs=pair, perf_mode=DR,
                             start=(g == 0), stop=(g == NG - 1),
                             skip_group_check=True)
            nc.tensor.matmul(sm[:], lhsT=env["ones8"][:], rhs=pair,
                             perf_mode=DR, start=(g == 0), stop=(g == NG - 1),
                             skip_group_check=True)

        for g in range(NG):
            dve_own = g in dve_pairs
            ps = psb.tile([P, 2, CH], F32, tag="sc", name=f"ps{ib}_{g}")
            for h in range(2):
                jc = 2 * g + h
                kslice = kT8[:, jc * P:(jc + 1) * P]
                nc.tensor.matmul(ps[:, h, :], lhsT=kslice, rhs=qblk,
                                 perf_mode=QK_PM, skip_group_check=True)
            if dve_own:
                nc.vector.tensor_scalar(PT[:, 2 * g:2 * g + 2, :].bitcast(I8),
                                        ps[:], SCHRAU_A * SCALE, SCHRAU_B,
                                        AOP.mult, AOP.add)
            run_extra(ib, g)
            if g > 3:
                emit_pv(g - 4)
            if not dve_own:
                nc.scalar.activation(PT[:, 2 * g:2 * g + 2, :], ps[:],
                                     AFT.Exp, scale=SCALE)
        emit_pv(NG - 4)
        emit_pv(NG - 3)
        emit_pv(NG - 2)
        emit_pv(NG - 1)

        # normalize and project
        recip = work.tile([P, CH], F32, tag="recip", name=f"recip{ib}")
        nc.vector.reciprocal_approx_fast(recip[:], sm[:])
        outn = work.tile([P, CH], F32R, tag="outn", name=f"outn{ib}")
        nc.vector.tensor_mul(outn[:], acc[:], recip[:])

        psp = pssum.tile([P, CH], F32, tag="sp", name=f"psp{ib}")
        nc.tensor.matmul(psp[:], lhsT=wp[:], rhs=outn[:])
        stage = outp.tile([P, CH], F32, tag="stage", name=f"stage{ib}")
        nc.vector.scalar_tensor_tensor(stage[:], psp[:], pbf[:, 0:1],
                                       x_sb[:, ts(ib, CH)], AOP.add, AOP.add)
        nc.gpsimd.dma_start(env["out_d"].ap()[:, ts(ib, CH)], stage[:])

    # any prologue steps not covered by PLACEMENT run at the rep's end
    if nxt is not None:
        for name, fn in nxt["steps"]:
            if name not in emitted:
                emitted.add(name)
                fn()


_NC_CACHE = {}


def _get_nc(reps=1):
    key = reps
    if key not in _NC_CACHE:
        _NC_CACHE[key] = _build_program(reps)
    return _NC_CACHE[key]


def _make_in_maps(x, gn_weight, gn_bias, qkv_weight, qkv_bias, proj_weight,
                  proj_bias):
    x = np.ascontiguousarray(x, dtype=np.float32)
    qkv_weight = np.asarray(qkv_weight, dtype=np.float32)
    qkv_bias = np.asarray(qkv_bias, dtype=np.float32)
    proj_weight = np.asarray(proj_weight, dtype=np.float32)
    proj_bias = np.asarray(proj_bias, dtype=np.float32)
    gn_weight = np.asarray(gn_weight, dtype=np.float32)
    gn_bias = np.asarray(gn_bias, dtype=np.float32)

    b = x.shape[0]
    xf = x.reshape(b, C, N)
    wqT = np.ascontiguousarray(qkv_weight[0:C].T)
    wkT = np.ascontiguousarray(qkv_weight[C:2 * C].T)
    wvT = np.ascontiguousarray(qkv_weight[2 * C:3 * C].T)
    wpT = np.ascontiguousarray(proj_weight.T)
    qkvb = np.ascontiguousarray(qkv_bias.reshape(3, C).T)
    pbv = np.ascontiguousarray(proj_bias.reshape(C, 1))
    gnwv = np.ascontiguousarray(gn_weight.reshape(C, 1))
    gnbv = np.ascontiguousarray(gn_bias.reshape(C, 1))

    in_maps = []
    for core in range(8):
        bi, half = core // 2, core % 2
        xc = xf[bi]
        if half == 1:  # own query half first; k/v order is irrelevant
            xc = np.concatenate([xc[:, NH:], xc[:, :NH]], axis=1)
        in_maps.append({
            "x": np.ascontiguousarray(xc),
            "wqT": wqT, "wkT": wkT, "wvT": wvT, "wpT": wpT,
            "qkvb": qkvb, "pb": pbv, "gnw": gnwv, "gnb": gnbv,
        })
    return in_maps


def run_on_cores(trace=False, reps=1, **inputs):
    """Build + run on the 8 cores; returns (BassKernelResults, output array)."""
    nc = _get_nc(reps)
    in_maps = _make_in_maps(**inputs)
    res = run_bass_kernel_spmd(nc, in_maps, core_ids=list(range(8)),
                               trace=trace)
    b = np.asarray(inputs["x"]).shape[0]
    h = w = 64
    out = np.empty((b, C, N), dtype=np.float32)
    for core in range(8):
        bi, half = core // 2, core % 2
        out[bi][:, half * NH:(half + 1) * NH] = res.results[core]["out"]
    return res, out.reshape(b, C, h, w)


def kernel(**inputs) -> np.ndarray:
    _, out = run_on_cores(trace=False, **inputs)
    return out


# revision 26
# speedup vs baseline: 1.7087x; 1.1663x over previous
# Trainium2 Bass kernel for nn_Attention_35433480192757
#
# reference computation (b=4, c=128, h=w=64, n=h*w=4096):
#   GroupNorm(8, c) -> 1x1 conv qkv -> full [n, n] attention per batch
#   -> 1x1 conv proj -> residual add
#
# Sharding: 8 cores = 4 batches x 2 query-row halves. Each core computes the
# full k/v for its batch (cheap: the qkv matmuls are tiny) and attention for
# its 2048 query rows. Host-side, each core's x is column-PERMUTED so that
# its own query half occupies columns 0:2048 -- attention is invariant to
# the j-enumeration order, and this keeps the SPMD program identical across
# cores with no separate xq input.
#
# Per-core strategy (fp8 + DoubleRow PV + two-engine softmax exp):
#   - x kept as [c=128 partitions, n] fp32; GroupNorm folded into the qkv
#     weights (xn = x*s_c + t_c per channel, computed on device; rsqrt via
#     a vector-engine bit-trick seed + Newton).
#   - q,k,v are produced as fp8e4. QK^T runs as plain fp8 contraction-128
#     matmuls. PV and the softmax-denominator (ones) matmuls consume P as
#     fp8 DoubleRow pairs.
#   - The k bias cancels in softmax; the v bias is folded into the proj
#     bias (pb' = pb + Wp @ bv).
#   - exp(scores) is written to fp8e5 by BOTH the scalar engine (true exp)
#     and the vector engine (Schraudolph bit-trick), split by a static
#     per-pair schedule.
#   - The whole per-rep prologue (GroupNorm stats, weight folds, k/q/v
#     production) is SOFTWARE-PIPELINED one rep ahead: its instructions are
#     emitted interleaved into the previous rep's attention pair loops, so
#     its PSUM tiles slot into the "sc"/"vv" rotations mid-stream and the
#     scalar engine never drains at rep boundaries.

import numpy as np
from contextlib import ExitStack

import concourse.bass as bass
from concourse import bacc
import concourse.tile as tile
import concourse.mybir as mybir
from concourse.bass import ts
from concourse.bass_utils import run_bass_kernel_spmd

P = 128          # partitions == channels
C = 128
N = 4096         # sequence length (h*w) per batch
NH = 2048        # query rows per core
CH = 512         # free-dim chunk (one PSUM bank of fp32)
NCH = N // CH    # 8 column chunks of x
NQCH = NH // CH  # 4 column chunks of q
NJC = N // P     # 32 key chunks (contraction over j)
NG = NJC // 2    # 16 j-chunk pairs per i-block
NIB = NH // CH   # 4 i-blocks per core
NUM_GROUPS = 8
GSIZE = C // NUM_GROUPS
EPS = 1e-5
SCALE = float(C) ** -0.5

F32 = mybir.dt.float32
F32R = mybir.dt.float32r
FP8 = mybir.dt.float8e4
FP8W = mybir.dt.float8e5   # P matrix: wide-range fp8 (e5m2)
I8 = mybir.dt.int8
AOP = mybir.AluOpType
AFT = mybir.ActivationFunctionType
DR = mybir.MatmulPerfMode.DoubleRow

# Schraudolph exp for fp8e5 (bias 15, 2 mantissa bits):
#   fp8e5_bits(exp(x)) ~= trunc(x * 4*log2e + 60 + c). e5m2's range covers
#   exp of +-10 sigma scores, so no clamping or shifting is needed; c=0.494
#   zeroes the mean multiplicative bias of the truncation.
SCHRAU_A = 4 * 1.4426950408889634
SCHRAU_B = 60.0 + 0.494

# Per-i-block sets of j-chunk pairs whose exp runs on the vector engine
# (Schraudolph); the rest use the scalar engine's exp.
DVE_EXP = {
    0: (1, 3, 5, 7, 9),
    1: (0, 2, 4, 6, 9, 11, 13),
    2: (1, 3, 5, 7, 9, 11, 13, 15),
    3: (2, 5, 8, 11, 14),
}

QK_PM = None
K_DVE = ()
import os as _os
if _os.environ.get("K_DVE"):
    K_DVE = tuple(int(v) for v in _os.environ["K_DVE"].split(",") if v != "")
if _os.environ.get("QK_DP"):
    QK_PM = mybir.MatmulPerfMode.DoublePixel
if _os.environ.get("DVE_EXP_CFG"):
    # e.g. "1,3,5,7,9,11,13|2,4,6,8,11,14|1,4,6,9,11,13|2,5,8,11,14"
    _parts = _os.environ["DVE_EXP_CFG"].split("|")
    DVE_EXP = {i: tuple(int(v) for v in p.split(",") if v != "")
               for i, p in enumerate(_parts)}


def _build_program(reps=1):
    nc = bacc.Bacc(trn_type="TRN2", num_devices=8)

    x_d = nc.dram_tensor("x", [P, N], F32R, kind="ExternalInput")
    wqT_d = nc.dram_tensor("wqT", [P, P], F32, kind="ExternalInput")
    wkT_d = nc.dram_tensor("wkT", [P, P], F32, kind="ExternalInput")
    wvT_d = nc.dram_tensor("wvT", [P, P], F32, kind="ExternalInput")
    wpT_d = nc.dram_tensor("wpT", [P, P], F32R, kind="ExternalInput")
    qkvb_d = nc.dram_tensor("qkvb", [P, 3], F32, kind="ExternalInput")
    pb_d = nc.dram_tensor("pb", [P, 1], F32, kind="ExternalInput")
    gnw_d = nc.dram_tensor("gnw", [P, 1], F32, kind="ExternalInput")
    gnb_d = nc.dram_tensor("gnb", [P, 1], F32, kind="ExternalInput")
    out_d = nc.dram_tensor("out", [P, NH], F32, kind="ExternalOutput")

    gmat_np = np.zeros((P, P), dtype=np.float32)
    for g in range(NUM_GROUPS):
        gmat_np[g * GSIZE:(g + 1) * GSIZE, g * GSIZE:(g + 1) * GSIZE] = 1.0 / GSIZE
    gmat_d = nc.inline_tensor(gmat_np, "gmat")

    with ExitStack() as ctx:
        tc = ctx.enter_context(tile.TileContext(nc))

        consts = ctx.enter_context(tc.tile_pool(name="consts", bufs=1))
        wfold = ctx.enter_context(tc.tile_pool(name="wfold", bufs=2))
        xpool = ctx.enter_context(tc.tile_pool(name="xpool", bufs=2))
        kqv = ctx.enter_context(tc.tile_pool(name="kqv", bufs=2))
        ptp = ctx.enter_context(tc.tile_pool(name="ptp", bufs=3))
        work = ctx.enter_context(tc.tile_pool(name="work", bufs=2))
        small = ctx.enter_context(tc.tile_pool(name="small", bufs=2))
        outp = ctx.enter_context(tc.tile_pool(name="outp", bufs=2))
        psb = ctx.enter_context(tc.tile_pool(name="psb", bufs=3, space="PSUM"))
        psv = psb
        psacc = ctx.enter_context(tc.tile_pool(name="psacc", bufs=1, space="PSUM"))
        pssum = ctx.enter_context(tc.tile_pool(name="pssum", bufs=1, space="PSUM"))

        env = dict(
            nc=nc, consts=consts, wfold=wfold, xpool=xpool, kqv=kqv, ptp=ptp,
            work=work, small=small, outp=outp, psb=psb, psv=psv, psacc=psacc,
            pssum=pssum, x_d=x_d, wqT_d=wqT_d, wkT_d=wkT_d, wvT_d=wvT_d,
            wpT_d=wpT_d, qkvb_d=qkvb_d, pb_d=pb_d, gnw_d=gnw_d, gnb_d=gnb_d,
            gmat_d=gmat_d, out_d=out_d,
        )

        _emit_consts(env)

        # rep 0 prologue emitted serially upfront (cold start).
        st0 = _prologue_steps(env, 0)
        for _, fn in st0["steps"]:
            fn()
        states = [st0]

        for r in range(reps):
            nxt = _prologue_steps(env, r + 1) if r + 1 < reps else None
            _emit_attention(env, states[r], nxt)
            if nxt is not None:
                states.append(nxt)

    nc.compile()
    return nc


def _emit_consts(env):
    """One-time loads: weights, biases, gmat, ones; PE warmup."""
    nc = env["nc"]
    consts = env["consts"]

    wq = consts.tile([P, P], F32, tag="wq", name="wq")
    nc.sync.dma_start(wq[:], env["wqT_d"].ap())
    wk = consts.tile([P, P], F32, tag="wk", name="wk")
    nc.sync.dma_start(wk[:], env["wkT_d"].ap())
    wv = consts.tile([P, P], F32, tag="wv", name="wv")
    nc.sync.dma_start(wv[:], env["wvT_d"].ap())
    wp = consts.tile([P, P], F32R, tag="wp", name="wp")
    nc.sync.dma_start(wp[:], env["wpT_d"].ap())
    qkvb = consts.tile([P, 3], F32, tag="qkvb", name="qkvb")
    nc.sync.dma_start(qkvb[:], env["qkvb_d"].ap())
    pb = consts.tile([P, 1], F32, tag="pb", name="pb")
    nc.sync.dma_start(pb[:], env["pb_d"].ap())
    gnw = consts.tile([P, 1], F32, tag="gnw", name="gnw")
    nc.sync.dma_start(gnw[:], env["gnw_d"].ap())
    gnb = consts.tile([P, 1], F32, tag="gnb", name="gnb")
    nc.sync.dma_start(gnb[:], env["gnb_d"].ap())
    ones8 = consts.tile([P, 2, P], FP8, tag="ones8", name="ones8")
    nc.gpsimd.memset(ones8[:], 1.0)
    # gmat last on the queue: the warmup matmul below absorbs the DMA-queue
    # semaphore wait once (walrus codegen allows only one sync-wait on a
    # self-loading fp32 matmul).
    gmat = consts.tile([P, P], F32, tag="gmat", name="gmat")
    nc.sync.dma_start(gmat[:], env["gmat_d"].ap())

    ps_w = env["psb"].tile([P, 8], F32, tag="sc", name="ps_warm")
    nc.tensor.matmul(ps_w[:, 0:2], lhsT=gmat[:], rhs=gmat[:, 0:2])

    env["wq"], env["wk"], env["wv"], env["wp"] = wq, wk, wv, wp
    env["qkvb"], env["pb"], env["gnw"], env["gnb"] = qkvb, pb, gnw, gnb
    env["gmat"], env["ones8"] = gmat, ones8


def _prologue_steps(env, rep):
    """Build the prologue for rep `rep` as a state dict + ordered list of
    emission callbacks (to run serially for rep 0, or interleaved into the
    previous rep's attention loops)."""
    nc = env["nc"]
    small, wfold, xpool, kqv = (env["small"], env["wfold"], env["xpool"],
                                env["kqv"])
    psb, psv = env["psb"], env["psv"]

    st = {}
    steps = []

    def add(name):
        def deco(fn):
            steps.append((name, fn))
            return fn
        return deco

    @add("xdma")
    def _():
        x_sb = xpool.tile([P, N], F32R, tag="x", name=f"x_sb{rep}")
        st["x"] = x_sb
        for s in range(NCH // 2):
            eng = nc.sync if s % 2 == 0 else nc.gpsimd
            eng.dma_start(x_sb[:, ts(s, 2 * CH)], env["x_d"].ap()[:, ts(s, 2 * CH)])
        st["stats"] = small.tile([P, NCH, 6], F32, tag="stats",
                                 name=f"stats{rep}")

    for _s in range(NCH):
        def _stats(s=_s):
            nc.vector.bn_stats(st["stats"][:, s, :], st["x"][:, ts(s, CH)])
        steps.append((f"stats{_s}", _stats))

    @add("aggr")
    def _():
        mv = small.tile([P, 2], F32, tag="mv", name=f"mv{rep}")
        nc.vector.bn_aggr(mv[:], st["stats"][:])
        # t2 = [mean_c, E[x^2]_c]; prep on the (idle) gpsimd engine so the
        # chain does not queue behind DVE exps.
        t2 = small.tile([P, 2], F32, tag="t2", name=f"t2{rep}")
        nc.vector.tensor_copy(t2[:, 0:1], mv[:, 0:1])
        nc.vector.scalar_tensor_tensor(t2[:, 1:2], mv[:, 0:1], mv[:, 0:1],
                                       mv[:, 1:2], AOP.mult, AOP.add)
        st["t2"] = t2

    @add("gnmm")
    def _():
        # group stats via block-diagonal averaging matrix; the tiny PSUM
        # tile borrows a slot of the "sc" pair rotation.
        ps_t = psb.tile([P, 8], F32, tag="sc", name=f"ps_gn{rep}")
        nc.tensor.matmul(ps_t[:, 0:2], lhsT=env["gmat"][:], rhs=st["t2"][:])
        gstat = small.tile([P, 2], F32, tag="gstat", name=f"gstat{rep}")
        nc.scalar.copy(gstat[:], ps_t[:, 0:2])
        st["gstat"] = gstat

    @add("rstd")
    def _():
        # whole chain on DVE in one batch (walrus allows no ALU ops on Pool)
        gstat = st["gstat"]
        varn = small.tile([P, 1], F32, tag="varn", name=f"varn{rep}")
        nc.vector.scalar_tensor_tensor(varn[:], gstat[:, 0:1], gstat[:, 0:1],
                                       gstat[:, 1:2], AOP.mult, AOP.subtract)
        vpos = small.tile([P, 1], F32, tag="vpos", name=f"vpos{rep}")
        nc.vector.tensor_scalar(vpos[:], varn[:], -1.0, EPS, AOP.mult, AOP.add)
        tsh = small.tile([P, 1], mybir.dt.int32, tag="tsh", name=f"tsh{rep}")
        nc.vector.tensor_scalar(tsh[:], vpos[:].bitcast(mybir.dt.int32), 1,
                                None, AOP.arith_shift_right)
        rstd = small.tile([P, 1], F32, tag="rstd", name=f"rstd{rep}")
        nc.vector.tensor_scalar(rstd[:].bitcast(mybir.dt.int32), tsh[:], -1,
                                0x5f3759df, AOP.mult, AOP.add)
        for it in range(2):
            nt = small.tile([P, 1], F32, tag="nt", name=f"nt{rep}_{it}")
            nc.vector.tensor_mul(nt[:], rstd[:], rstd[:])
            nc.vector.tensor_mul(nt[:], nt[:], vpos[:])
            nc.vector.tensor_scalar(nt[:], nt[:], -0.5, 1.5, AOP.mult, AOP.add)
            nc.vector.tensor_mul(rstd[:], rstd[:], nt[:])
        s_c = small.tile([P, 1], F32, tag="s_c", name=f"s_c{rep}")
        nc.vector.tensor_mul(s_c[:], rstd[:], env["gnw"][:])
        # t_n = mean_g*s_c - gn_bias = -t_c
        t_n = small.tile([P, 1], F32, tag="t_n", name=f"t_n{rep}")
        nc.vector.scalar_tensor_tensor(t_n[:], gstat[:, 0:1], s_c[:],
                                       env["gnb"][:], AOP.mult, AOP.subtract)
        st["s_c"], st["t_n"] = s_c, t_n

    @add("foldq")
    def _():
        wq_s = wfold.tile([P, P], F32R, tag="wq_s", name=f"wq_s{rep}")
        nc.vector.tensor_scalar_mul(wq_s[:], env["wq"][:], st["s_c"][:])
        st["wq_s"] = wq_s

    @add("foldk")
    def _():
        wk_s = wfold.tile([P, P], F32R, tag="wk_s", name=f"wk_s{rep}")
        nc.vector.tensor_scalar_mul(wk_s[:], env["wk"][:], st["s_c"][:])
        st["wk_s"] = wk_s

    @add("foldv")
    def _():
        wv_s = wfold.tile([P, 2, P], F32R, tag="wv_s", name=f"wv_s{rep}")
        nc.vector.tensor_scalar_mul(wv_s[:, 0, :], env["wv"][:], st["s_c"][:])
        nc.vector.tensor_scalar_mul(wv_s[:, 1, :], env["wv"][:], st["s_c"][:])
        st["wv_s"] = wv_s

    @add("bias")
    def _():
        # q bias (k bias cancels; v bias folds into the proj bias below)
        ps_b = psb.tile([P, 8], F32, tag="sc", name=f"ps_b{rep}")
        nc.tensor.matmul(ps_b[:, 0:1], lhsT=env["wq"][:], rhs=st["t_n"][:])
        nc.tensor.matmul(ps_b[:, 1:2], lhsT=env["wv"][:], rhs=st["t_n"][:])
        bq = small.tile([P, 1], F32, tag="bq", name=f"bq{rep}")
        nc.scalar.activation(bq[:], ps_b[:, 0:1], AFT.Identity,
                             bias=env["qkvb"][:, 0:1], scale=-1.0)
        bv = small.tile([P, 1], F32, tag="bv", name=f"bv{rep}")
        nc.scalar.activation(bv[:], ps_b[:, 1:2], AFT.Identity,
                             bias=env["qkvb"][:, 2:3], scale=-1.0)
        st["bq"], st["bv"] = bq, bv

    @add("pbf")
    def _():
        ps_p = psb.tile([P, 8], F32, tag="sc", name=f"ps_p{rep}")
        nc.tensor.matmul(ps_p[:, 0:1], lhsT=env["wp"][:].bitcast(F32),
                         rhs=st["bv"][:])
        pbf = small.tile([P, 1], F32, tag="pbf", name=f"pbf{rep}")
        nc.scalar.activation(pbf[:], ps_p[:, 0:1], AFT.Identity,
                             bias=env["pb"][:], scale=1.0)
        st["pbf"] = pbf
        st["kT8"] = kqv.tile([P, N], FP8, tag="kdr", name=f"kT8{rep}")
        st["qT8"] = kqv.tile([P, NH], FP8, tag="qdr", name=f"qT8{rep}")
        st["vnat"] = kqv.tile([P, NJC, P], FP8, tag="vnat", name=f"vnat{rep}")

    for _s in range(NCH // 2):
        def _k(s=_s):
            # two 512-col matmuls into one 2-bank tile; ONE 1024-col copy
            pk = psb.tile([P, 2, CH], F32, tag="sc", name=f"psk{rep}_{s}")
            for h in range(2):
                nc.tensor.matmul(pk[:, h, :], lhsT=st["wk_s"][:],
                                 rhs=st["x"][:, ts(2 * s + h, CH)])
            if s in K_DVE:
                nc.vector.tensor_copy(st["kT8"][:, ts(s, 2 * CH)], pk[:])
            else:
                nc.scalar.copy(st["kT8"][:, ts(s, 2 * CH)], pk[:])
        steps.append((f"k{_s}", _k))

    for _s in range(NQCH // 2):
        def _q(s=_s):
            # q carries the bias: scalar-engine activation applies it
            pq = psb.tile([P, 2, CH], F32, tag="sc", name=f"psq{rep}_{s}")
            for h in range(2):
                nc.tensor.matmul(pq[:, h, :], lhsT=st["wq_s"][:],
                                 rhs=st["x"][:, ts(2 * s + h, CH)])
            nc.scalar.activation(st["qT8"][:, ts(s, 2 * CH)], pq[:],
                                 AFT.Identity, bias=st["bq"][:], scale=1.0)
        steps.append((f"q{_s}", _q))

    for _g in range(NJC // 4):
        def _v(q8=_g):
            # four chunks per PSUM tile, each duplicated twice (free=256
            # keeps fp32r at full rate); one copy reads the first replicas.
            pv = psv.tile([P, 4, 2, P], F32, tag="sc", name=f"psv{rep}_{q8}")
            for h in range(4):
                jc = 4 * q8 + h
                nc.tensor.matmul(pv[:, h, :, :],
                                 lhsT=st["x"][:, jc * P:(jc + 1) * P],
                                 rhs=st["wv_s"][:])
            nc.vector.tensor_copy(st["vnat"][:, 4 * q8:4 * q8 + 4, :],
                                  pv[:, :, 0, :])
        steps.append((f"v{_g}", _v))

    st["steps"] = steps
    return st


# Placement of next-rep prologue steps inside the current rep's attention:
# {(ib, pair_g): [step names]}. Steps not listed here run at their default
# position (appended after the pair loop of the listed block).
def _placement():
    pl = {}
    pl[(0, None)] = ["xdma"]                     # x DMA at ib0 entry
    # stats: all 8 chunks over ib0's back half (no DVE exps there)
    for i in range(NCH):
        pl[(0, 8 + i)] = [f"stats{i}"]
    pl[(1, 1)] = ["aggr"]
    pl[(1, 3)] = ["gnmm"]
    pl[(1, 5)] = ["rstd"]
    pl[(1, 8)] = ["foldq", "foldk"]
    pl[(1, 10)] = ["foldv"]
    pl[(1, 12)] = ["bias"]
    pl[(1, 14)] = ["pbf"]
    for i in range(NCH // 2):                     # k production over ib2
        pl[(2, 4 + 2 * i)] = [f"k{i}"]
    for i in range(NQCH // 2):                    # q production at ib2 end
        pl[(2, 12 + 2 * i)] = [f"q{i}"]
    # v production: 8 calls over ib3 pairs 0..14 (one every other pair)
    for i in range(NJC // 4):
        pl.setdefault((3, 2 * i), []).append(f"v{i}")
    return pl


PLACEMENT = _placement()


def _emit_attention(env, st, nxt):
    nc = env["nc"]
    ptp, work, outp = env["ptp"], env["work"], env["outp"]
    psb, psv, psacc, pssum = env["psb"], env["psv"], env["psacc"], env["pssum"]

    nxt_steps = dict(nxt["steps"]) if nxt is not None else {}
    emitted = set()

    def run_extra(ib, g):
        for name in PLACEMENT.get((ib, g), []) or []:
            fn = nxt_steps.get(name)
            if fn is not None and name not in emitted:
                emitted.add(name)
                fn()

    x_sb, kT8, qT8, vnat = st["x"], st["kT8"], st["qT8"], st["vnat"]
    wp, pbf = env["wp"], st["pbf"]

    for ib in range(NIB):
        if nxt is not None:
            run_extra(ib, None)
        PT = ptp.tile([P, NJC, CH], FP8W, tag="PT", name=f"PT{ib}")
        acc = psacc.tile([P, CH], F32, tag="acc", name=f"acc{ib}")
        sm = pssum.tile([P, CH], F32, tag="sp", name=f"sm{ib}")
        qblk = qT8[:, ts(ib, CH)]
        dve_pairs = DVE_EXP[ib]

        def emit_pv(g):
            pair = PT[:, 2 * g:2 * g + 2, :]
            nc.tensor.matmul(acc[:], lhsT=vnat[:, 2 * g:2 * g + 2, :],
                             rhs=pair, perf_mode=DR,
                             start=(g == 0), stop=(g == NG - 1),
                             skip_group_check=True)
            nc.tensor.matmul(sm[:], lhsT=env["ones8"][:], rhs=pair,
                             perf_mode=DR, start=(g == 0), stop=(g == NG - 1),
                             skip_group_check=True)

        for g in range(NG):
            dve_own = g in dve_pairs
            ps = psb.tile([P, 2, CH], F32, tag="sc", name=f"ps{ib}_{g}")
            for h in range(2):
                jc = 2 * g + h
                kslice = kT8[:, jc * P:(jc + 1) * P]
                nc.tensor.matmul(ps[:, h, :], lhsT=kslice, rhs=qblk,
                                 perf_mode=QK_PM, skip_group_check=True)
            if dve_own:
                nc.vector.tensor_scalar(PT[:, 2 * g:2 * g + 2, :].bitcast(I8),
                                        ps[:], SCHRAU_A * SCALE, SCHRAU_B,
                                        AOP.mult, AOP.add)
            run_extra(ib, g)
            if g > 3:
                emit_pv(g - 4)
            if not dve_own:
                nc.scalar.activation(PT[:, 2 * g:2 * g + 2, :], ps[:],
                                     AFT.Exp, scale=SCALE)
        emit_pv(NG - 4)
        emit_pv(NG - 3)
        emit_pv(NG - 2)
        emit_pv(NG - 1)

        # normalize and project
        recip = work.tile([P, CH], F32, tag="recip", name=f"recip{ib}")
        nc.vector.reciprocal_approx_fast(recip[:], sm[:])
        outn = work.tile([P, CH], F32R, tag="outn", name=f"outn{ib}")
        nc.vector.tensor_mul(outn[:], acc[:], recip[:])

        psp = pssum.tile([P, CH], F32, tag="sp", name=f"psp{ib}")
        nc.tensor.matmul(psp[:], lhsT=wp[:], rhs=outn[:])
        stage = outp.tile([P, CH], F32, tag="stage", name=f"stage{ib}")
        nc.vector.scalar_tensor_tensor(stage[:], psp[:], pbf[:, 0:1],
                                       x_sb[:, ts(ib, CH)], AOP.add, AOP.add)
        nc.gpsimd.dma_start(env["out_d"].ap()[:, ts(ib, CH)], stage[:])

    # any prologue steps not covered by PLACEMENT run at the rep's end
    if nxt is not None:
        for name, fn in nxt["steps"]:
            if name not in emitted:
                emitted.add(name)
                fn()


_NC_CACHE = {}


def _get_nc(reps=1):
    key = reps
    if key not in _NC_CACHE:
        _NC_CACHE[key] = _build_program(reps)
    return _NC_CACHE[key]


def _make_in_maps(x, gn_weight, gn_bias, qkv_weight, qkv_bias, proj_weight,
                  proj_bias):
    x = np.ascontiguousarray(x, dtype=np.float32)
    qkv_weight = np.asarray(qkv_weight, dtype=np.float32)
    qkv_bias = np.asarray(qkv_bias, dtype=np.float32)
    proj_weight = np.asarray(proj_weight, dtype=np.float32)
    proj_bias = np.asarray(proj_bias, dtype=np.float32)
    gn_weight = np.asarray(gn_weight, dtype=np.float32)
    gn_bias = np.asarray(gn_bias, dtype=np.float32)

    b = x.shape[0]
    xf = x.reshape(b, C, N)
    wqT = np.ascontiguousarray(qkv_weight[0:C].T)
    wkT = np.ascontiguousarray(qkv_weight[C:2 * C].T)
    wvT = np.ascontiguousarray(qkv_weight[2 * C:3 * C].T)
    wpT = np.ascontiguousarray(proj_weight.T)
    qkvb = np.ascontiguousarray(qkv_bias.reshape(3, C).T)
    pbv = np.ascontiguousarray(proj_bias.reshape(C, 1))
    gnwv = np.ascontiguousarray(gn_weight.reshape(C, 1))
    gnbv = np.ascontiguousarray(gn_bias.reshape(C, 1))

    in_maps = []
    for core in range(8):
        bi, half = core // 2, core % 2
        xc = xf[bi]
        if half == 1:  # own query half first; k/v order is irrelevant
            xc = np.concatenate([xc[:, NH:], xc[:, :NH]], axis=1)
        in_maps.append({
            "x": np.ascontiguousarray(xc),
            "wqT": wqT, "wkT": wkT, "wvT": wvT, "wpT": wpT,
            "qkvb": qkvb, "pb": pbv, "gnw": gnwv, "gnb": gnbv,
        })
    return in_maps


def run_on_cores(trace=False, reps=1, **inputs):
    """Build + run on the 8 cores; returns (BassKernelResults, output array)."""
    nc = _get_nc(reps)
    in_maps = _make_in_maps(**inputs)
    res = run_bass_kernel_spmd(nc, in_maps, core_ids=list(range(8)),
                               trace=trace)
    b = np.asarray(inputs["x"]).shape[0]
    h = w = 64
    out = np.empty((b, C, N), dtype=np.float32)
    for core in range(8):
        bi, half = core // 2, core % 2
        out[bi][:, half * NH:(half + 1) * NH] = res.results[core]["out"]
    return res, out.reshape(b, C, h, w)


def kernel(**inputs) -> np.ndarray:
    _, out = run_on_cores(trace=False, **inputs)
    return out
